# revision 11
# baseline (speedup 1.0000x reference)
"""Trainium2 Bass kernel for nn_DepthwiseXCorr (SiamRPN++-style depthwise-xcorr head).

Pipeline per sample (data-parallel over batch: 64 samples -> 8 cores x 8):
  conv3x3(kernel,wk)+BN+ReLU -> k_feat [256,5,5]
  conv3x3(search,ws)+BN+ReLU -> s_feat [256,29,29]
  depthwise xcorr(s_feat,k_feat) -> feat [256,25,25]
  1x1 conv w1 + BN + ReLU -> h [256,25,25]
  1x1 conv w2 + b2 -> out [20,25,25]

Convolutions run on the PE in bf16 (inputs/weights quantized host-side; the
accumulation stays fp32 in PSUM). The depthwise xcorr is spread over four
engines: a few taps on the PE (per-tap diagonal-weight matmuls into PSUM),
the Activation engine bridges the PSUM partial into SBUF, then the Pool and
DVE engines chain scalar_tensor_tensor multiply-accumulates in place.
"""
import numpy as np

EPS = 1e-5
NCORES = 8
BPC = 8          # samples per core

# per-(sample, og) tap split: (n_pe, n_act_pool, n_dve) summing to 25.
# n_pe taps run on the PE (diag matmuls, diags made in batches of 5 on DVE,
# so n_pe must be a multiple of 5); n_act_pool taps are computed as products
# on the Activation engine and accumulated by Pool tensor_tensor adds; the
# rest chain scalar_tensor_tensor MACs on the DVE.
SCHED = [[(5, 6, 14), (5, 7, 13)] for _ in range(BPC)]

_CACHE = {}


def _shift_window(ap_2d, base_off, rows, cols, rowstride):
    """AP reading [128, rows, cols] window at element offset base_off of a
    [128, W] SBUF view, row stride in elements."""
    import concourse.bass as bass
    return bass.AP(
        tensor=ap_2d.tensor,
        offset=ap_2d.offset + base_off,
        ap=[list(ap_2d.ap[0]), [rowstride, rows], [1, cols]],
    )


def _build(bench_R=0):
    import concourse.bacc as bacc
    import concourse.bass as bass
    import concourse.mybir as mybir
    import concourse.tile as tile

    f32 = mybir.dt.float32
    f32r = mybir.dt.float32r
    bf16 = mybir.dt.bfloat16
    AF = mybir.ActivationFunctionType
    ALU = mybir.AluOpType

    nc = bacc.Bacc("TRN2", target_bir_lowering=False, debug=False,
                   num_devices=NCORES)

    search_d = nc.declare_dram_parameter("search", [BPC, 128, 2, 968], bf16, isOutput=False)
    tmpl_d = nc.declare_dram_parameter("tmpl", [128, 2, BPC, 52], bf16, isOutput=False)
    wkt_d = nc.declare_dram_parameter("wkt", [128, 36, 128], bf16, isOutput=False)
    wst_d = nc.declare_dram_parameter("wst", [128, 36, 128], bf16, isOutput=False)
    w1t_d = nc.declare_dram_parameter("w1t", [128, 4, 128], f32r, isOutput=False)
    w2t_d = nc.declare_dram_parameter("w2t", [128, 2, 20], f32r, isOutput=False)
    bnk_d = nc.declare_dram_parameter("bnk", [128, 4], f32, isOutput=False)
    bns_d = nc.declare_dram_parameter("bns", [128, 4], f32, isOutput=False)
    bnh_d = nc.declare_dram_parameter("bnh", [128, 4], f32, isOutput=False)
    b2_d = nc.declare_dram_parameter("b2t", [128, 1], f32, isOutput=False)
    id_d = nc.declare_dram_parameter("ident", [128, 128], f32, isOutput=False)
    out_d = nc.declare_dram_parameter("out", [BPC, 20, 625], f32, isOutput=True)

    def tidx(cig, dy, dx, og):
        return ((cig * 3 + dy) * 3 + dx) * 2 + og

    with tile.TileContext(nc) as tc:
        with (
            tc.tile_pool(name="wp", bufs=1) as wp,
            tc.tile_pool(name="sp", bufs=3) as sp,
            tc.tile_pool(name="fp", bufs=3) as fp,
            tc.tile_pool(name="dp", bufs=6) as dp,
            tc.tile_pool(name="mp", bufs=8) as mp,
            tc.tile_pool(name="psc", bufs=4, space="PSUM") as psc,
            tc.tile_pool(name="psx", bufs=2, space="PSUM") as psx,
            tc.tile_pool(name="psh", bufs=2, space="PSUM") as psh,
        ):
            wkt = wp.tile([128, 36, 128], bf16)
            wst = wp.tile([128, 36, 128], bf16)
            w1t = wp.tile([128, 4, 128], f32r)
            w2t = wp.tile([128, 2, 20], f32r)
            bnk = wp.tile([128, 4], f32)
            bns = wp.tile([128, 4], f32)
            bnh = wp.tile([128, 4], f32)
            b2t = wp.tile([128, 1], f32)
            ident = wp.tile([128, 128], f32)
            k_in = wp.tile([128, 2, BPC, 52], bf16)
            nc.gpsimd.dma_start(out=k_in, in_=tmpl_d[:, :, :, :])
            nc.gpsimd.dma_start(out=bnk, in_=bnk_d[:, :])
            nc.gpsimd.dma_start(out=wkt, in_=wkt_d[:, :, :])
            nc.scalar.dma_start(out=wst, in_=wst_d[:, :, :])
            nc.sync.dma_start(out=w1t, in_=w1t_d[:, :, :])
            nc.sync.dma_start(out=w2t, in_=w2t_d[:, :, :])
            nc.gpsimd.dma_start(out=bns, in_=bns_d[:, :])
            nc.gpsimd.dma_start(out=bnh, in_=bnh_d[:, :])
            nc.gpsimd.dma_start(out=b2t, in_=b2_d[:, :])
            nc.gpsimd.dma_start(out=ident, in_=id_d[:, :])

            # conv_kernel branch: all samples batched, N = 8*5*5 = 200
            k_feat = wp.tile([128, 2, BPC * 25], f32)
            for og in range(2):
                pk = psc.tile([128, 512], f32, tag="conv")
                j = 0
                for cig in range(2):
                    for dy in range(3):
                        for dx in range(3):
                            base = k_in[:, cig, :, :]
                            rhs = bass.AP(tensor=base.tensor,
                                          offset=base.offset + dy * 7 + dx,
                                          ap=[list(base.ap[0]), [52, BPC], [7, 5], [1, 5]])
                            nc.tensor.matmul(pk[:, :200], wkt[:, tidx(cig, dy, dx, og), :],
                                             rhs, start=(j == 0), stop=(j == 17))
                            j += 1
                nc.scalar.activation(k_feat[:, og, :], pk[:, :200], AF.Relu,
                                     scale=bnk[:, og:og + 1], bias=bnk[:, 2 + og:3 + og])

            taps = [(t // 5, t % 5) for t in range(25)]

            import contextlib
            loop_cm = (tc.For_i(0, bench_R, 1,
                                  hint_engines=(mybir.EngineType.PE,
                                                mybir.EngineType.DVE,
                                                mybir.EngineType.Activation))
                         if bench_R else contextlib.nullcontext())
            with loop_cm:
              for s in range(BPC):
                  s_in = sp.tile([128, 2, 968], bf16, tag="s_in")
                  nc.sync.dma_start(out=s_in, in_=search_d[s, :, :, :])

                  # conv_search: out plane 29 rows x 29 cols, packed stride 29
                  s_feat = sp.tile([128, 2, 841], f32r, tag="s_feat")
                  for og in range(2):
                      for off, y0c, rws in ((0, 0, 17), (493, 17, 12)):
                          w = rws * 29
                          pc = psc.tile([128, 512], f32, tag="conv")
                          j = 0
                          for cig in range(2):
                              for dy in range(3):
                                  for dx in range(3):
                                      rhs = _shift_window(s_in[:, cig, :], (y0c + dy) * 31 + dx,
                                                          rws, 29, 31)
                                      nc.tensor.matmul(pc[:, :w], wst[:, tidx(cig, dy, dx, og), :],
                                                       rhs, start=(j == 0), stop=(j == 17))
                                      j += 1
                          nc.scalar.activation(s_feat[:, og, off:off + w], pc[:, :w], AF.Relu,
                                               scale=bns[:, og:og + 1], bias=bns[:, 2 + og:3 + og])

                  # depthwise xcorr, two independent partials per og:
                  #   A: PE diag taps -> PSUM -> Act bridge -> DVE stt chain -> featr (f32r)
                  #   B: Act tap products, Pool tensor_tensor add chain -> fpr (f32r)
                  # the head matmul accumulates both partials in PSUM.
                  featc = fp.tile([128, 2, 625], f32, tag="featc")
                  featr = fp.tile([128, 2, 640], f32r, tag="featr")
                  fpool = fp.tile([128, 2, 625], f32, tag="fpool")
                  fpr = fp.tile([128, 2, 640], f32r, tag="fpr")
                  nc.gpsimd.memset(featr[:, :, 625:640].bitcast(f32), 0.0)
                  nc.gpsimd.memset(fpr[:, :, 625:640].bitcast(f32), 0.0)
                  for og in range(2):
                      npe, nap, ndve = SCHED[s][og]
                      assert npe % 5 == 0 and npe >= 5
                      pe_taps = taps[:npe]
                      act_taps = taps[npe:npe + nap]
                      dve_taps = taps[npe + nap:]
                      sf = s_feat[:, og, :]
                      kf = k_feat[:, og, :]
                      kbase = s * 25

                      # diag batches on DVE: dg[:, i, :] = ident * k[tap i]
                      dlist = []
                      for dy in range(npe // 5):
                          dg = dp.tile([128, 5, 128], f32r, tag="diag")
                          id_b = bass.AP(tensor=ident[:, :].tensor, offset=ident[:, :].offset,
                                         ap=[list(ident[:, :].ap[0]), [0, 5], [1, 128]])
                          k_b = bass.AP(tensor=kf.tensor, offset=kf.offset + kbase + dy * 5,
                                        ap=[list(kf.ap[0]), [1, 5], [0, 128]])
                          nc.vector.tensor_tensor(dg, id_b, k_b, ALU.mult)
                          for dx in range(5):
                              dlist.append(dg[:, dx, :])

                      # PE partial: diag-weight matmuls accumulated in PSUM.
                      # fp32r matmul needs even innermost counts -> 26-wide
                      # windows; tap (4,4) would read past s_feat so PE taps
                      # must come from the row-major prefix (dy<4).
                      assert all(dy < 4 for dy, dx in pe_taps)
                      for y0, rows in ((0, 13), (13, 12)):
                          n = rows * 26
                          px = psx.tile([128, 338], f32, tag="x")
                          for i, (dy, dx) in enumerate(pe_taps):
                              rhs = _shift_window(sf, (y0 + dy) * 29 + dx, rows, 26, 29)
                              nc.tensor.matmul(px[:, :n], dlist[i], rhs,
                                               start=(i == 0), stop=(i == npe - 1))
                          # bridge PSUM partial into SBUF (Act engine)
                          src_px = _shift_window(px, 0, rows, 25, 26)
                          dst_f = featc[:, og, y0 * 25: y0 * 25 + rows * 25].rearrange(
                              "p (r c) -> p r c", c=25)
                          nc.scalar.activation(dst_f, src_px, AF.Copy)

                      fv = featc[:, og, :].rearrange("p (r c) -> p r c", c=25)
                      frv = featr[:, og, 0:625].rearrange("p (r c) -> p r c", c=25)
                      gv = fpool[:, og, :].rearrange("p (r c) -> p r c", c=25)
                      grv = fpr[:, og, 0:625].rearrange("p (r c) -> p r c", c=25)
                      # partial B: Act products, Pool add chain (first product
                      # seeds fpool, last add writes the f32r tile)
                      assert len(act_taps) == 0 or len(act_taps) >= 2
                      for j, (dy, dx) in enumerate(act_taps):
                          win = _shift_window(sf, dy * 29 + dx, 25, 25, 29)
                          kap = kf[:, kbase + dy * 5 + dx: kbase + dy * 5 + dx + 1]
                          if j == 0:
                              nc.scalar.activation(gv, win, AF.Copy, scale=kap)
                          else:
                              m = mp.tile([128, 25, 25], f32, tag="m")
                              nc.scalar.activation(m, win, AF.Copy, scale=kap)
                              dst = grv if j == len(act_taps) - 1 else gv
                              nc.gpsimd.tensor_tensor(dst, gv, m, ALU.add)
                      # partial A: DVE chain; last tap writes the f32r tile
                      assert len(dve_taps) >= 1
                      for j, (dy, dx) in enumerate(dve_taps):
                          win = _shift_window(sf, dy * 29 + dx, 25, 25, 29)
                          kap = kf[:, kbase + dy * 5 + dx: kbase + dy * 5 + dx + 1]
                          dst = frv if j == len(dve_taps) - 1 else fv
                          nc.vector.scalar_tensor_tensor(dst, win, kap, fv, ALU.mult, ALU.add)

                  # head: 1x1 conv -> BN -> ReLU -> 1x1 conv + b2
                  # (accumulates the live xcorr partials per input og)
                  h = fp.tile([128, 2, 640], f32r, tag="h")
                  for og in range(2):
                      srcs = []
                      for ogi in range(2):
                          srcs.append((ogi, featr))
                          if SCHED[s][ogi][1] > 0:
                              srcs.append((ogi, fpr))
                      for off, w in ((0, 320), (320, 306)):
                          ph = psh.tile([128, 320], f32, tag="h")
                          for j, (ogi, part) in enumerate(srcs):
                              nc.tensor.matmul(ph[:, :w], w1t[:, ogi * 2 + og, :],
                                               part[:, ogi, off:off + w],
                                               start=(j == 0), stop=(j == len(srcs) - 1))
                          nc.scalar.activation(h[:, og, off:off + w], ph[:, :w], AF.Relu,
                                               scale=bnh[:, og:og + 1], bias=bnh[:, 2 + og:3 + og])

                  out_s = fp.tile([128, 640], f32, tag="outs")
                  for off, w in ((0, 320), (320, 306)):
                      po = psh.tile([128, 320], f32, tag="h")
                      nc.tensor.matmul(po[0:20, :w], w2t[:, 0, :], h[:, 0, off:off + w],
                                       start=True, stop=False)
                      nc.tensor.matmul(po[0:20, :w], w2t[:, 1, :], h[:, 1, off:off + w],
                                       start=False, stop=True)
                      nc.scalar.activation(out_s[0:20, off:off + w], po[0:20, :w],
                                           AF.Identity, bias=b2t[0:20, 0:1])
                  nc.sync.dma_start(out=out_d[s, :, :], in_=out_s[0:20, 0:625])

    nc.compile()
    return nc


def _pack(inputs):
    f32 = np.float32
    try:
        import ml_dtypes
        bf16 = ml_dtypes.bfloat16
    except ImportError:
        import jax.numpy as jnp
        bf16 = jnp.bfloat16
    kern = np.ascontiguousarray(inputs["kernel"], dtype=f32)
    search = np.ascontiguousarray(inputs["search"], dtype=f32)
    wk, ws = inputs["wk"].astype(f32), inputs["ws"].astype(f32)
    w1, w2, b2 = inputs["w1"].astype(f32), inputs["w2"].astype(f32), inputs["b2"].astype(f32)

    def fold(scale, bias, mean, var):
        inv = scale.astype(f32) / np.sqrt(var.astype(f32) + EPS)
        sh = bias.astype(f32) - mean.astype(f32) * inv
        arr = np.zeros((128, 4), f32)
        arr[:, 0:2] = inv.reshape(2, 128).T
        arr[:, 2:4] = sh.reshape(2, 128).T
        return arr

    bnk = fold(inputs["bnk_scale"], inputs["bnk_bias"], inputs["bnk_mean"], inputs["bnk_var"])
    bns = fold(inputs["bns_scale"], inputs["bns_bias"], inputs["bns_mean"], inputs["bns_var"])
    bnh = fold(inputs["bnh_scale"], inputs["bnh_bias"], inputs["bnh_mean"], inputs["bnh_var"])

    # conv weights -> lhsT tiles [ci, (cig,dy,dx,og), co]
    def conv_w(w):
        w6 = w.reshape(2, 128, 2, 128, 3, 3)           # og co cig ci dy dx
        return np.ascontiguousarray(
            w6.transpose(3, 2, 4, 5, 0, 1).reshape(128, 36, 128).astype(bf16))

    wkt, wst = conv_w(wk), conv_w(ws)
    w1t = np.ascontiguousarray(
        w1[:, :, 0, 0].reshape(2, 128, 2, 128).transpose(3, 2, 0, 1).reshape(128, 4, 128))
    w2t = np.ascontiguousarray(
        w2[:, :, 0, 0].reshape(20, 2, 128).transpose(2, 1, 0))
    b2t = np.zeros((128, 1), f32)
    b2t[:20, 0] = b2
    ident = np.eye(128, dtype=f32)

    # search [64,256,31,31] -> per core [8, 128(ci), 2(cig), 961]
    sr = np.zeros((NCORES, BPC, 128, 2, 968), bf16)
    sr[..., :961] = search.reshape(NCORES, BPC, 2, 128, 961).transpose(0, 1, 3, 2, 4).astype(bf16)
    # kernel [64,256,7,7] -> per core [128(ci), 2(cig), 8(s), 49]
    kr = np.zeros((NCORES, 128, 2, BPC, 52), bf16)
    kr[..., :49] = kern.reshape(NCORES, BPC, 2, 128, 49).transpose(0, 3, 2, 1, 4).astype(bf16)

    in_maps = []
    for c in range(NCORES):
        in_maps.append({
            "search": np.ascontiguousarray(sr[c]),
            "tmpl": np.ascontiguousarray(kr[c]),
            "wkt": wkt, "wst": wst, "w1t": w1t, "w2t": w2t,
            "bnk": bnk, "bns": bns, "bnh": bnh, "b2t": b2t, "ident": ident,
        })
    return in_maps


def get_program(bench_R=0):
    key = f"nc{bench_R}"
    if key not in _CACHE:
        _CACHE[key] = _build(bench_R)
    return _CACHE[key]


def kernel(**inputs):
    from concourse.bass_utils import run_bass_kernel_spmd
    nc = get_program()
    in_maps = _pack(inputs)
    res = run_bass_kernel_spmd(nc, in_maps, core_ids=list(range(NCORES)))
    out = np.stack([res.results[c]["out"] for c in range(NCORES)], axis=0)
    return out.reshape(64, 20, 25, 25).astype(np.float32)


# revision 16
# speedup vs baseline: 1.1853x; 1.1853x over previous
"""Trainium2 Bass kernel for nn_DepthwiseXCorr (SiamRPN++-style depthwise-xcorr head).

Pipeline per sample (data-parallel over batch: 64 samples -> 8 cores x 8):
  conv3x3(kernel,wk)+BN+ReLU -> k_feat [256,5,5]
  conv3x3(search,ws)+BN+ReLU -> s_feat [256,29,29]
  depthwise xcorr(s_feat,k_feat) -> feat [256,25,25]
  1x1 conv w1 + BN + ReLU -> h [256,25,25]
  1x1 conv w2 + b2 -> out [20,25,25]

Convolutions run on the PE in bf16 (inputs/weights quantized host-side; the
accumulation stays fp32 in PSUM). The depthwise xcorr is spread over four
engines: a few taps on the PE (per-tap diagonal-weight matmuls into PSUM),
the Activation engine bridges the PSUM partial into SBUF, then the Pool and
DVE engines chain scalar_tensor_tensor multiply-accumulates in place.
"""
import numpy as np

EPS = 1e-5
NCORES = 8
BPC = 8          # samples per core

# per-(sample, og) tap split: (n_pe, n_act_pool, n_dve) summing to 25.
# n_pe taps run on the PE (diag matmuls, diags made in batches of 5 on DVE,
# so n_pe must be a multiple of 5); n_act_pool taps are computed as products
# on the Activation engine and accumulated by Pool tensor_tensor adds; the
# rest chain scalar_tensor_tensor MACs on the DVE.
SCHED = [[(5, 7, 13), (10, 8, 7)] for _ in range(BPC - 1)] + [[(20, 0, 5), (20, 0, 5)]]
CK200 = True      # conv_kernel via 4D AP at N=200 (else N=288 windows)
DMAQ = 0          # 0: all weights on sync; 1: spread across queues
MERGE_B = 2       # 2: merge partial B into featc with a DVE add (head reads 1 partial)
WSPLIT = True     # split wkt/wst DMAs across sync+gpsimd queues

_CACHE = {}


def _shift_window(ap_2d, base_off, rows, cols, rowstride):
    """AP reading [128, rows, cols] window at element offset base_off of a
    [128, W] SBUF view, row stride in elements."""
    import concourse.bass as bass
    return bass.AP(
        tensor=ap_2d.tensor,
        offset=ap_2d.offset + base_off,
        ap=[list(ap_2d.ap[0]), [rowstride, rows], [1, cols]],
    )


def _build(bench_R=0):
    import concourse.bacc as bacc
    import concourse.bass as bass
    import concourse.mybir as mybir
    import concourse.tile as tile

    f32 = mybir.dt.float32
    f32r = mybir.dt.float32r
    bf16 = mybir.dt.bfloat16
    AF = mybir.ActivationFunctionType
    ALU = mybir.AluOpType

    nc = bacc.Bacc("TRN2", target_bir_lowering=False, debug=False,
                   num_devices=NCORES)

    search_d = nc.declare_dram_parameter("search", [BPC, 128, 2, 968], bf16, isOutput=False)
    tmpl_d = nc.declare_dram_parameter("tmpl", [128, 2, BPC, 52], bf16, isOutput=False)
    wkt_d = nc.declare_dram_parameter("wkt", [128, 36, 128], bf16, isOutput=False)
    wst_d = nc.declare_dram_parameter("wst", [128, 36, 128], bf16, isOutput=False)
    w1t_d = nc.declare_dram_parameter("w1t", [128, 4, 128], f32r, isOutput=False)
    w2t_d = nc.declare_dram_parameter("w2t", [128, 2, 20], f32r, isOutput=False)
    bnk_d = nc.declare_dram_parameter("bnk", [128, 4], f32, isOutput=False)
    bns_d = nc.declare_dram_parameter("bns", [128, 4], f32, isOutput=False)
    bnh_d = nc.declare_dram_parameter("bnh", [128, 4], f32, isOutput=False)
    b2_d = nc.declare_dram_parameter("b2t", [128, 1], f32, isOutput=False)
    id_d = nc.declare_dram_parameter("ident", [128, 128], f32, isOutput=False)
    out_d = nc.declare_dram_parameter("out", [BPC, 20, 625], f32, isOutput=True)

    def tidx(cig, dy, dx, og):
        return ((cig * 3 + dy) * 3 + dx) * 2 + og

    with tile.TileContext(nc) as tc:
        with (
            tc.tile_pool(name="wp", bufs=1) as wp,
            tc.tile_pool(name="sp", bufs=3) as sp,
            tc.tile_pool(name="fp", bufs=3) as fp,
            tc.tile_pool(name="dp", bufs=6) as dp,
            tc.tile_pool(name="mp", bufs=8) as mp,
            tc.tile_pool(name="psc", bufs=4, space="PSUM") as psc,
            tc.tile_pool(name="psx", bufs=2, space="PSUM") as psx,
            tc.tile_pool(name="psh", bufs=2, space="PSUM") as psh,
        ):
            wkt = wp.tile([128, 36, 128], bf16)
            wst = wp.tile([128, 36, 128], bf16)
            w1t = wp.tile([128, 4, 128], f32r)
            w2t = wp.tile([128, 2, 20], f32r)
            bnk = wp.tile([128, 4], f32)
            bns = wp.tile([128, 4], f32)
            bnh = wp.tile([128, 4], f32)
            b2t = wp.tile([128, 1], f32)
            ident = wp.tile([128, 128], f32)
            k_in = wp.tile([128, 2, BPC, 52], bf16)
            nc.gpsimd.dma_start(out=k_in, in_=tmpl_d[:, :, :, :])
            nc.gpsimd.dma_start(out=bnk, in_=bnk_d[:, :])
            if WSPLIT:
                nc.sync.dma_start(out=wkt[:, 0:18, :], in_=wkt_d[:, 0:18, :])
                nc.gpsimd.dma_start(out=wkt[:, 18:36, :], in_=wkt_d[:, 18:36, :])
                nc.sync.dma_start(out=wst[:, 0:18, :], in_=wst_d[:, 0:18, :])
                nc.gpsimd.dma_start(out=wst[:, 18:36, :], in_=wst_d[:, 18:36, :])
            elif DMAQ == 1:
                nc.gpsimd.dma_start(out=wkt, in_=wkt_d[:, :, :])
                nc.scalar.dma_start(out=wst, in_=wst_d[:, :, :])
            else:
                nc.sync.dma_start(out=wkt, in_=wkt_d[:, :, :])
                nc.sync.dma_start(out=wst, in_=wst_d[:, :, :])
            nc.sync.dma_start(out=w1t, in_=w1t_d[:, :, :])
            nc.sync.dma_start(out=w2t, in_=w2t_d[:, :, :])
            nc.gpsimd.dma_start(out=bns, in_=bns_d[:, :])
            nc.gpsimd.dma_start(out=bnh, in_=bnh_d[:, :])
            nc.gpsimd.dma_start(out=b2t, in_=b2_d[:, :])
            nc.gpsimd.dma_start(out=ident, in_=id_d[:, :])

            # conv_kernel branch: all samples batched
            kfs = 25 if CK200 else 36
            k_feat = wp.tile([128, 2, BPC * kfs], f32)
            for og in range(2):
                pk = psc.tile([128, 512], f32, tag="conv")
                j = 0
                for cig in range(2):
                    for dy in range(3):
                        for dx in range(3):
                            base = k_in[:, cig, :, :]
                            if CK200:
                                rhs = bass.AP(tensor=base.tensor,
                                              offset=base.offset + dy * 7 + dx,
                                              ap=[list(base.ap[0]), [52, BPC], [7, 5], [1, 5]])
                            else:
                                rhs = base[:, :, dy * 7 + dx: dy * 7 + dx + 36]
                            nc.tensor.matmul(pk[:, :BPC * kfs], wkt[:, tidx(cig, dy, dx, og), :],
                                             rhs, start=(j == 0), stop=(j == 17))
                            j += 1
                nc.scalar.activation(k_feat[:, og, :], pk[:, :BPC * kfs], AF.Relu,
                                     scale=bnk[:, og:og + 1], bias=bnk[:, 2 + og:3 + og])

            taps = [(t // 5, t % 5) for t in range(25)]

            import contextlib
            loop_cm = (tc.For_i(0, bench_R, 1,
                                  hint_engines=(mybir.EngineType.PE,
                                                mybir.EngineType.DVE,
                                                mybir.EngineType.Activation))
                         if bench_R else contextlib.nullcontext())
            with loop_cm:
              for s in range(BPC):
                  s_in = sp.tile([128, 2, 968], bf16, tag="s_in")
                  nc.sync.dma_start(out=s_in, in_=search_d[s, :, :, :])

                  # conv_search: out plane 29 rows x 29 cols, packed stride 29
                  s_feat = sp.tile([128, 2, 841], f32r, tag="s_feat")
                  for og in range(2):
                      for off, y0c, rws in ((0, 0, 17), (493, 17, 12)):
                          w = rws * 29
                          pc = psc.tile([128, 512], f32, tag="conv")
                          j = 0
                          for cig in range(2):
                              for dy in range(3):
                                  for dx in range(3):
                                      rhs = _shift_window(s_in[:, cig, :], (y0c + dy) * 31 + dx,
                                                          rws, 29, 31)
                                      nc.tensor.matmul(pc[:, :w], wst[:, tidx(cig, dy, dx, og), :],
                                                       rhs, start=(j == 0), stop=(j == 17))
                                      j += 1
                          nc.scalar.activation(s_feat[:, og, off:off + w], pc[:, :w], AF.Relu,
                                               scale=bns[:, og:og + 1], bias=bns[:, 2 + og:3 + og])

                  # depthwise xcorr, two independent partials per og:
                  #   A: PE diag taps -> PSUM -> Act bridge -> DVE stt chain -> featr (f32r)
                  #   B: Act tap products, Pool tensor_tensor add chain -> fpr (f32r)
                  # the head matmul accumulates both partials in PSUM.
                  featc = fp.tile([128, 2, 625], f32, tag="featc")
                  featr = fp.tile([128, 2, 640], f32r, tag="featr")
                  fpool = fp.tile([128, 2, 625], f32, tag="fpool")
                  fpr = fp.tile([128, 2, 640], f32r, tag="fpr")
                  nc.gpsimd.memset(featr[:, :, 625:640].bitcast(f32), 0.0)
                  nc.gpsimd.memset(fpr[:, :, 625:640].bitcast(f32), 0.0)
                  for og in range(2):
                      npe, nap, ndve = SCHED[s][og]
                      assert npe % 5 == 0 and npe >= 5
                      pe_taps = taps[:npe]
                      act_taps = taps[npe:npe + nap]
                      dve_taps = taps[npe + nap:]
                      sf = s_feat[:, og, :]
                      kf = k_feat[:, og, :]
                      krs2 = 5 if CK200 else 7
                      kbase = s * kfs

                      # diag batches on DVE: dg[:, i, :] = ident * k[tap i]
                      dlist = []
                      for dy in range(npe // 5):
                          dg = dp.tile([128, 5, 128], f32r, tag="diag")
                          id_b = bass.AP(tensor=ident[:, :].tensor, offset=ident[:, :].offset,
                                         ap=[list(ident[:, :].ap[0]), [0, 5], [1, 128]])
                          krs = 5 if CK200 else 7
                          k_b = bass.AP(tensor=kf.tensor, offset=kf.offset + kbase + dy * krs,
                                        ap=[list(kf.ap[0]), [1, 5], [0, 128]])
                          nc.vector.tensor_tensor(dg, id_b, k_b, ALU.mult)
                          for dx in range(5):
                              dlist.append(dg[:, dx, :])

                      # PE partial: diag-weight matmuls accumulated in PSUM.
                      # fp32r matmul needs even innermost counts -> 26-wide
                      # windows; tap (4,4) would read past s_feat so PE taps
                      # must come from the row-major prefix (dy<4).
                      assert all(dy < 4 for dy, dx in pe_taps)
                      for y0, rows in ((0, 13), (13, 12)):
                          n = rows * 26
                          px = psx.tile([128, 338], f32, tag="x")
                          for i, (dy, dx) in enumerate(pe_taps):
                              rhs = _shift_window(sf, (y0 + dy) * 29 + dx, rows, 26, 29)
                              nc.tensor.matmul(px[:, :n], dlist[i], rhs,
                                               start=(i == 0), stop=(i == npe - 1))
                          # bridge PSUM partial into SBUF (Act engine)
                          src_px = _shift_window(px, 0, rows, 25, 26)
                          dst_f = featc[:, og, y0 * 25: y0 * 25 + rows * 25].rearrange(
                              "p (r c) -> p r c", c=25)
                          nc.scalar.activation(dst_f, src_px, AF.Copy)

                      fv = featc[:, og, :].rearrange("p (r c) -> p r c", c=25)
                      frv = featr[:, og, 0:625].rearrange("p (r c) -> p r c", c=25)
                      gv = fpool[:, og, :].rearrange("p (r c) -> p r c", c=25)
                      grv = fpr[:, og, 0:625].rearrange("p (r c) -> p r c", c=25)
                      # partial B: Act products, Pool add chain; without
                      # MERGE_B the last add writes the f32r tile fpr (second
                      # head partial), with MERGE_B it stays in fpool and a
                      # final Pool add folds it into featc before the last
                      # DVE tap.
                      assert len(act_taps) == 0 or len(act_taps) >= 2
                      for j, (dy, dx) in enumerate(act_taps):
                          win = _shift_window(sf, dy * 29 + dx, 25, 25, 29)
                          kap = kf[:, kbase + dy * krs2 + dx: kbase + dy * krs2 + dx + 1]
                          if j == 0:
                              nc.scalar.activation(gv, win, AF.Copy, scale=kap)
                          else:
                              m = mp.tile([128, 25, 25], f32, tag="m")
                              nc.scalar.activation(m, win, AF.Copy, scale=kap)
                              dst = gv if MERGE_B or j < len(act_taps) - 1 else grv
                              nc.gpsimd.tensor_tensor(dst, gv, m, ALU.add)
                      # partial A: DVE chain; last tap writes the f32r tile
                      assert len(dve_taps) >= 1
                      for j, (dy, dx) in enumerate(dve_taps):
                          if MERGE_B and act_taps and j == len(dve_taps) - 1:
                              if MERGE_B == 2:
                                  nc.vector.tensor_tensor(fv, fv, gv, ALU.add)
                              else:
                                  nc.gpsimd.tensor_tensor(fv, fv, gv, ALU.add)
                          win = _shift_window(sf, dy * 29 + dx, 25, 25, 29)
                          kap = kf[:, kbase + dy * krs2 + dx: kbase + dy * krs2 + dx + 1]
                          dst = frv if j == len(dve_taps) - 1 else fv
                          nc.vector.scalar_tensor_tensor(dst, win, kap, fv, ALU.mult, ALU.add)

                  # head: 1x1 conv -> BN -> ReLU -> 1x1 conv + b2
                  # (accumulates the live xcorr partials per input og)
                  h = fp.tile([128, 2, 640], f32r, tag="h")
                  for og in range(2):
                      srcs = []
                      for ogi in range(2):
                          srcs.append((ogi, featr))
                          if SCHED[s][ogi][1] > 0 and not MERGE_B:
                              srcs.append((ogi, fpr))
                      for off, w in ((0, 320), (320, 306)):
                          ph = psh.tile([128, 320], f32, tag="h")
                          for j, (ogi, part) in enumerate(srcs):
                              nc.tensor.matmul(ph[:, :w], w1t[:, ogi * 2 + og, :],
                                               part[:, ogi, off:off + w],
                                               start=(j == 0), stop=(j == len(srcs) - 1))
                          nc.scalar.activation(h[:, og, off:off + w], ph[:, :w], AF.Relu,
                                               scale=bnh[:, og:og + 1], bias=bnh[:, 2 + og:3 + og])

                  out_s = fp.tile([128, 640], f32, tag="outs")
                  for off, w in ((0, 320), (320, 306)):
                      po = psh.tile([128, 320], f32, tag="h")
                      nc.tensor.matmul(po[0:20, :w], w2t[:, 0, :], h[:, 0, off:off + w],
                                       start=True, stop=False)
                      nc.tensor.matmul(po[0:20, :w], w2t[:, 1, :], h[:, 1, off:off + w],
                                       start=False, stop=True)
                      nc.scalar.activation(out_s[0:20, off:off + w], po[0:20, :w],
                                           AF.Identity, bias=b2t[0:20, 0:1])
                  nc.sync.dma_start(out=out_d[s, :, :], in_=out_s[0:20, 0:625])

    nc.compile()
    return nc


def _pack(inputs):
    f32 = np.float32
    try:
        import ml_dtypes
        bf16 = ml_dtypes.bfloat16
    except ImportError:
        import jax.numpy as jnp
        bf16 = jnp.bfloat16
    kern = np.ascontiguousarray(inputs["kernel"], dtype=f32)
    search = np.ascontiguousarray(inputs["search"], dtype=f32)
    wk, ws = inputs["wk"].astype(f32), inputs["ws"].astype(f32)
    w1, w2, b2 = inputs["w1"].astype(f32), inputs["w2"].astype(f32), inputs["b2"].astype(f32)

    def fold(scale, bias, mean, var):
        inv = scale.astype(f32) / np.sqrt(var.astype(f32) + EPS)
        sh = bias.astype(f32) - mean.astype(f32) * inv
        arr = np.zeros((128, 4), f32)
        arr[:, 0:2] = inv.reshape(2, 128).T
        arr[:, 2:4] = sh.reshape(2, 128).T
        return arr

    bnk = fold(inputs["bnk_scale"], inputs["bnk_bias"], inputs["bnk_mean"], inputs["bnk_var"])
    bns = fold(inputs["bns_scale"], inputs["bns_bias"], inputs["bns_mean"], inputs["bns_var"])
    bnh = fold(inputs["bnh_scale"], inputs["bnh_bias"], inputs["bnh_mean"], inputs["bnh_var"])

    # conv weights -> lhsT tiles [ci, (cig,dy,dx,og), co]
    def conv_w(w):
        w6 = w.reshape(2, 128, 2, 128, 3, 3)           # og co cig ci dy dx
        return np.ascontiguousarray(
            w6.transpose(3, 2, 4, 5, 0, 1).reshape(128, 36, 128).astype(bf16))

    wkt, wst = conv_w(wk), conv_w(ws)
    w1t = np.ascontiguousarray(
        w1[:, :, 0, 0].reshape(2, 128, 2, 128).transpose(3, 2, 0, 1).reshape(128, 4, 128))
    w2t = np.ascontiguousarray(
        w2[:, :, 0, 0].reshape(20, 2, 128).transpose(2, 1, 0))
    b2t = np.zeros((128, 1), f32)
    b2t[:20, 0] = b2
    ident = np.eye(128, dtype=f32)

    # search [64,256,31,31] -> per core [8, 128(ci), 2(cig), 961]
    sr = np.zeros((NCORES, BPC, 128, 2, 968), bf16)
    sr[..., :961] = search.reshape(NCORES, BPC, 2, 128, 961).transpose(0, 1, 3, 2, 4).astype(bf16)
    # kernel [64,256,7,7] -> per core [128(ci), 2(cig), 8(s), 49]
    kr = np.zeros((NCORES, 128, 2, BPC, 52), bf16)
    kr[..., :49] = kern.reshape(NCORES, BPC, 2, 128, 49).transpose(0, 3, 2, 1, 4).astype(bf16)

    in_maps = []
    for c in range(NCORES):
        in_maps.append({
            "search": np.ascontiguousarray(sr[c]),
            "tmpl": np.ascontiguousarray(kr[c]),
            "wkt": wkt, "wst": wst, "w1t": w1t, "w2t": w2t,
            "bnk": bnk, "bns": bns, "bnh": bnh, "b2t": b2t, "ident": ident,
        })
    return in_maps


def get_program(bench_R=0):
    key = f"nc{bench_R}"
    if key not in _CACHE:
        _CACHE[key] = _build(bench_R)
    return _CACHE[key]


def kernel(**inputs):
    from concourse.bass_utils import run_bass_kernel_spmd
    nc = get_program()
    in_maps = _pack(inputs)
    res = run_bass_kernel_spmd(nc, in_maps, core_ids=list(range(NCORES)))
    out = np.stack([res.results[c]["out"] for c in range(NCORES)], axis=0)
    return out.reshape(64, 20, 25, 25).astype(np.float32)


# revision 27
# speedup vs baseline: 1.2237x; 1.0324x over previous
"""Trainium2 Bass kernel for nn_DepthwiseXCorr (SiamRPN++-style depthwise-xcorr head).

Pipeline per sample (data-parallel over batch: 64 samples -> 8 cores x 8):
  conv3x3(kernel,wk)+BN+ReLU -> k_feat [256,5,5]
  conv3x3(search,ws)+BN+ReLU -> s_feat [256,29,29]
  depthwise xcorr(s_feat,k_feat) -> feat [256,25,25]
  1x1 conv w1 + BN + ReLU -> h [256,25,25]
  1x1 conv w2 + b2 -> out [20,25,25]

Convolutions run on the PE in bf16 (inputs/weights quantized host-side; the
accumulation stays fp32 in PSUM). The depthwise xcorr is spread over four
engines: a few taps on the PE (per-tap diagonal-weight matmuls into PSUM),
the Activation engine bridges the PSUM partial into SBUF, then the Pool and
DVE engines chain scalar_tensor_tensor multiply-accumulates in place.
"""
import numpy as np

EPS = 1e-5
NCORES = 8
BPC = 8          # samples per core

# per-(sample, og) tap split: (n_pe, n_act_pool, n_dve) summing to 25.
# n_pe taps run on the PE (diag matmuls, diags made in batches of 5 on DVE,
# so n_pe must be a multiple of 5); n_act_pool taps are computed as products
# on the Activation engine and accumulated by Pool tensor_tensor adds; the
# rest chain scalar_tensor_tensor MACs on the DVE.
SCHED = [[(5, 7, 13), (8, 8, 9)] for _ in range(BPC - 1)] + [[(24, 0, 1), (24, 0, 1)]]
CK200 = True      # conv_kernel via 4D AP at N=200 (else N=288 windows)
DMAQ = 0          # 0: all weights on sync; 1: spread across queues
MERGE_B = 2       # 2: merge partial B into featc with a DVE add (head reads 1 partial)
WSPLIT = True     # split wkt/wst DMAs across sync+gpsimd queues
SEED_PSUM = True  # DVE chain seeds by reading px from PSUM (no Act bridges)
DIAG_ACT = 1      # og < DIAG_ACT gets per-diag creation on Act instead of DVE batch
PSC, PSX, PSH = 2, 2, 4
SPB, FPB, DPB, MPB = 3, 3, 6, 8
SINQ = 0          # 0: s_in DMA on sync queue; 1: on scalar (Act) queue

_CACHE = {}


def _shift_window(ap_2d, base_off, rows, cols, rowstride):
    """AP reading [128, rows, cols] window at element offset base_off of a
    [128, W] SBUF view, row stride in elements."""
    import concourse.bass as bass
    return bass.AP(
        tensor=ap_2d.tensor,
        offset=ap_2d.offset + base_off,
        ap=[list(ap_2d.ap[0]), [rowstride, rows], [1, cols]],
    )


def _build(bench_R=0):
    import concourse.bacc as bacc
    import concourse.bass as bass
    import concourse.mybir as mybir
    import concourse.tile as tile

    f32 = mybir.dt.float32
    f32r = mybir.dt.float32r
    bf16 = mybir.dt.bfloat16
    AF = mybir.ActivationFunctionType
    ALU = mybir.AluOpType

    nc = bacc.Bacc("TRN2", target_bir_lowering=False, debug=False,
                   num_devices=NCORES)

    search_d = nc.declare_dram_parameter("search", [BPC, 128, 2, 968], bf16, isOutput=False)
    tmpl_d = nc.declare_dram_parameter("tmpl", [128, 2, BPC, 52], bf16, isOutput=False)
    wkt_d = nc.declare_dram_parameter("wkt", [128, 36, 128], bf16, isOutput=False)
    wst_d = nc.declare_dram_parameter("wst", [128, 36, 128], bf16, isOutput=False)
    w1t_d = nc.declare_dram_parameter("w1t", [128, 4, 128], f32r, isOutput=False)
    w2t_d = nc.declare_dram_parameter("w2t", [128, 2, 20], f32r, isOutput=False)
    bnk_d = nc.declare_dram_parameter("bnk", [128, 4], f32, isOutput=False)
    bns_d = nc.declare_dram_parameter("bns", [128, 4], f32, isOutput=False)
    bnh_d = nc.declare_dram_parameter("bnh", [128, 4], f32, isOutput=False)
    b2_d = nc.declare_dram_parameter("b2t", [128, 1], f32, isOutput=False)
    id_d = nc.declare_dram_parameter("ident", [128, 128], f32, isOutput=False)
    out_d = nc.declare_dram_parameter("out", [BPC, 20, 625], f32, isOutput=True)

    def tidx(cig, dy, dx, og):
        return ((cig * 3 + dy) * 3 + dx) * 2 + og

    with tile.TileContext(nc) as tc:
        with (
            tc.tile_pool(name="wp", bufs=1) as wp,
            tc.tile_pool(name="sp", bufs=SPB) as sp,
            tc.tile_pool(name="fp", bufs=FPB) as fp,
            tc.tile_pool(name="dp", bufs=DPB) as dp,
            tc.tile_pool(name="mp", bufs=MPB) as mp,
            tc.tile_pool(name="psc", bufs=PSC, space="PSUM") as psc,
            tc.tile_pool(name="psx", bufs=PSX, space="PSUM") as psx,
            tc.tile_pool(name="psh", bufs=PSH, space="PSUM") as psh,
        ):
            wkt = wp.tile([128, 36, 128], bf16)
            wst = wp.tile([128, 36, 128], bf16)
            w1t = wp.tile([128, 4, 128], f32r)
            w2t = wp.tile([128, 2, 20], f32r)
            bnk = wp.tile([128, 4], f32)
            bns = wp.tile([128, 4], f32)
            bnh = wp.tile([128, 4], f32)
            b2t = wp.tile([128, 1], f32)
            ident = wp.tile([128, 128], f32)
            k_in = wp.tile([128, 2, BPC, 52], bf16)
            nc.gpsimd.dma_start(out=k_in, in_=tmpl_d[:, :, :, :])
            nc.gpsimd.dma_start(out=bnk, in_=bnk_d[:, :])
            if WSPLIT:
                nc.sync.dma_start(out=wkt[:, 0:18, :], in_=wkt_d[:, 0:18, :])
                nc.gpsimd.dma_start(out=wkt[:, 18:36, :], in_=wkt_d[:, 18:36, :])
                nc.sync.dma_start(out=wst[:, 0:18, :], in_=wst_d[:, 0:18, :])
                nc.gpsimd.dma_start(out=wst[:, 18:36, :], in_=wst_d[:, 18:36, :])
            elif DMAQ == 1:
                nc.gpsimd.dma_start(out=wkt, in_=wkt_d[:, :, :])
                nc.scalar.dma_start(out=wst, in_=wst_d[:, :, :])
            else:
                nc.sync.dma_start(out=wkt, in_=wkt_d[:, :, :])
                nc.sync.dma_start(out=wst, in_=wst_d[:, :, :])
            nc.sync.dma_start(out=w1t, in_=w1t_d[:, :, :])
            nc.sync.dma_start(out=w2t, in_=w2t_d[:, :, :])
            nc.gpsimd.dma_start(out=bns, in_=bns_d[:, :])
            nc.gpsimd.dma_start(out=bnh, in_=bnh_d[:, :])
            nc.gpsimd.dma_start(out=b2t, in_=b2_d[:, :])
            nc.gpsimd.dma_start(out=ident, in_=id_d[:, :])

            # conv_kernel branch: all samples batched
            kfs = 25 if CK200 else 36
            k_feat = wp.tile([128, 2, BPC * kfs], f32)
            for og in range(2):
                pk = psc.tile([128, 512], f32, tag="conv")
                j = 0
                for cig in range(2):
                    for dy in range(3):
                        for dx in range(3):
                            base = k_in[:, cig, :, :]
                            if CK200:
                                rhs = bass.AP(tensor=base.tensor,
                                              offset=base.offset + dy * 7 + dx,
                                              ap=[list(base.ap[0]), [52, BPC], [7, 5], [1, 5]])
                            else:
                                rhs = base[:, :, dy * 7 + dx: dy * 7 + dx + 36]
                            nc.tensor.matmul(pk[:, :BPC * kfs], wkt[:, tidx(cig, dy, dx, og), :],
                                             rhs, start=(j == 0), stop=(j == 17))
                            j += 1
                nc.scalar.activation(k_feat[:, og, :], pk[:, :BPC * kfs], AF.Relu,
                                     scale=bnk[:, og:og + 1], bias=bnk[:, 2 + og:3 + og])

            taps = [(t // 5, t % 5) for t in range(25)]

            import contextlib
            loop_cm = (tc.For_i(0, bench_R, 1,
                                  hint_engines=(mybir.EngineType.PE,
                                                mybir.EngineType.DVE,
                                                mybir.EngineType.Activation))
                         if bench_R else contextlib.nullcontext())
            with loop_cm:
              for s in range(BPC):
                  s_in = sp.tile([128, 2, 968], bf16, tag="s_in")
                  if SINQ:
                      nc.scalar.dma_start(out=s_in, in_=search_d[s, :, :, :])
                  else:
                      nc.sync.dma_start(out=s_in, in_=search_d[s, :, :, :])

                  # conv_search: out plane 29 rows x 29 cols, packed stride 29
                  s_feat = sp.tile([128, 2, 841], f32r, tag="s_feat")
                  for og in range(2):
                      for off, y0c, rws in ((0, 0, 17), (493, 17, 12)):
                          w = rws * 29
                          pc = psc.tile([128, 512], f32, tag="conv")
                          j = 0
                          for cig in range(2):
                              for dy in range(3):
                                  for dx in range(3):
                                      rhs = _shift_window(s_in[:, cig, :], (y0c + dy) * 31 + dx,
                                                          rws, 29, 31)
                                      nc.tensor.matmul(pc[:, :w], wst[:, tidx(cig, dy, dx, og), :],
                                                       rhs, start=(j == 0), stop=(j == 17))
                                      j += 1
                          nc.scalar.activation(s_feat[:, og, off:off + w], pc[:, :w], AF.Relu,
                                               scale=bns[:, og:og + 1], bias=bns[:, 2 + og:3 + og])

                  # depthwise xcorr, two independent partials per og:
                  #   A: PE diag taps -> PSUM -> Act bridge -> DVE stt chain -> featr (f32r)
                  #   B: Act tap products, Pool tensor_tensor add chain -> fpr (f32r)
                  # the head matmul accumulates both partials in PSUM.
                  featc = fp.tile([128, 2, 625], f32, tag="featc")
                  featr = fp.tile([128, 2, 640], f32r, tag="featr")
                  fpool = fp.tile([128, 2, 625], f32, tag="fpool")
                  fpr = fp.tile([128, 2, 640], f32r, tag="fpr")
                  nc.gpsimd.memset(featr[:, :, 625:640].bitcast(f32), 0.0)
                  nc.gpsimd.memset(fpr[:, :, 625:640].bitcast(f32), 0.0)
                  for og in range(2):
                      npe, nap, ndve = SCHED[s][og]
                      assert 1 <= npe <= 24
                      pe_taps = taps[:npe]
                      act_taps = taps[npe:npe + nap]
                      dve_taps = taps[npe + nap:]
                      sf = s_feat[:, og, :]
                      kf = k_feat[:, og, :]
                      krs2 = 5 if CK200 else 7
                      kbase = s * kfs

                      # diag batches on DVE: dg[:, i, :] = ident * k[tap i]
                      # (or per-diag Act creation when og < DIAG_ACT)
                      krs = 5 if CK200 else 7
                      dlist = []
                      if og < DIAG_ACT:
                          for (dy, dx) in pe_taps:
                              dga = dp.tile([128, 128], f32r, tag="diaga")
                              nc.scalar.activation(
                                  dga, ident, AF.Copy,
                                  scale=kf[:, kbase + dy * krs + dx: kbase + dy * krs + dx + 1])
                              dlist.append(dga)
                      else:
                          done = 0
                          for dy in range((npe + 4) // 5):
                              nb = min(5, npe - done)
                              dg = dp.tile([128, 5, 128], f32r, tag="diag")
                              id_b = bass.AP(tensor=ident[:, :].tensor, offset=ident[:, :].offset,
                                             ap=[list(ident[:, :].ap[0]), [0, nb], [1, 128]])
                              k_b = bass.AP(tensor=kf.tensor, offset=kf.offset + kbase + dy * krs,
                                            ap=[list(kf.ap[0]), [1, nb], [0, 128]])
                              nc.vector.tensor_tensor(dg[:, 0:nb, :], id_b, k_b, ALU.mult)
                              for i in range(nb):
                                  dlist.append(dg[:, i, :])
                              done += nb

                      # PE partial: diag-weight matmuls accumulated in PSUM.
                      # fp32r matmul needs even innermost counts -> 26-wide
                      # windows; tap (4,4) would read past s_feat so PE taps
                      # must come from the row-major prefix (dy<4).
                      assert all((dy, dx) != (4, 4) for dy, dx in pe_taps)
                      pxs = []
                      for y0, rows in ((0, 13), (13, 12)):
                          n = rows * 26
                          px = psx.tile([128, 338], f32, tag="x")
                          for i, (dy, dx) in enumerate(pe_taps):
                              rhs = _shift_window(sf, (y0 + dy) * 29 + dx, rows, 26, 29)
                              nc.tensor.matmul(px[:, :n], dlist[i], rhs,
                                               start=(i == 0), stop=(i == npe - 1))
                          pxs.append((y0, rows, px))
                          if not SEED_PSUM:
                              # bridge PSUM partial into SBUF (Act engine)
                              src_px = _shift_window(px, 0, rows, 25, 26)
                              dst_f = featc[:, og, y0 * 25: y0 * 25 + rows * 25].rearrange(
                                  "p (r c) -> p r c", c=25)
                              nc.scalar.activation(dst_f, src_px, AF.Copy)

                      fv = featc[:, og, :].rearrange("p (r c) -> p r c", c=25)
                      frv = featr[:, og, 0:625].rearrange("p (r c) -> p r c", c=25)
                      gv = fpool[:, og, :].rearrange("p (r c) -> p r c", c=25)
                      grv = fpr[:, og, 0:625].rearrange("p (r c) -> p r c", c=25)
                      # partial B: Act products, Pool add chain; without
                      # MERGE_B the last add writes the f32r tile fpr (second
                      # head partial), with MERGE_B it stays in fpool and a
                      # final Pool add folds it into featc before the last
                      # DVE tap.
                      assert len(act_taps) == 0 or len(act_taps) >= 2
                      for j, (dy, dx) in enumerate(act_taps):
                          win = _shift_window(sf, dy * 29 + dx, 25, 25, 29)
                          kap = kf[:, kbase + dy * krs2 + dx: kbase + dy * krs2 + dx + 1]
                          if j == 0:
                              nc.scalar.activation(gv, win, AF.Copy, scale=kap)
                          else:
                              m = mp.tile([128, 25, 25], f32, tag="m")
                              nc.scalar.activation(m, win, AF.Copy, scale=kap)
                              dst = gv if MERGE_B or j < len(act_taps) - 1 else grv
                              nc.gpsimd.tensor_tensor(dst, gv, m, ALU.add)
                      # partial A: DVE chain; last tap writes the f32r tile
                      assert len(dve_taps) >= 1
                      for j, (dy, dx) in enumerate(dve_taps):
                          if MERGE_B and act_taps and j == len(dve_taps) - 1:
                              if MERGE_B == 2:
                                  nc.vector.tensor_tensor(fv, fv, gv, ALU.add)
                              else:
                                  nc.gpsimd.tensor_tensor(fv, fv, gv, ALU.add)
                          kap = kf[:, kbase + dy * krs2 + dx: kbase + dy * krs2 + dx + 1]
                          if SEED_PSUM and j == 0:
                              # seed: two stts, each adding a px PSUM slice; if
                              # this is also the last tap, write featr directly
                              # (requires no B partial to merge)
                              last = j == len(dve_taps) - 1
                              assert not (last and act_taps)
                              base = featr if last else featc
                              for y0, rows, px in pxs:
                                  winp = _shift_window(sf, (dy + y0) * 29 + dx, rows, 25, 29)
                                  pxv = _shift_window(px, 0, rows, 25, 26)
                                  dstp = base[:, og, y0 * 25: y0 * 25 + rows * 25].rearrange(
                                      "p (r c) -> p r c", c=25)
                                  nc.vector.scalar_tensor_tensor(dstp, winp, kap, pxv,
                                                                 ALU.mult, ALU.add)
                              continue
                          win = _shift_window(sf, dy * 29 + dx, 25, 25, 29)
                          dst = frv if j == len(dve_taps) - 1 else fv
                          nc.vector.scalar_tensor_tensor(dst, win, kap, fv, ALU.mult, ALU.add)

                  # head: 1x1 conv -> BN -> ReLU -> 1x1 conv + b2
                  # (accumulates the live xcorr partials per input og)
                  h = fp.tile([128, 2, 640], f32r, tag="h")
                  for og in range(2):
                      srcs = []
                      for ogi in range(2):
                          srcs.append((ogi, featr))
                          if SCHED[s][ogi][1] > 0 and not MERGE_B:
                              srcs.append((ogi, fpr))
                      for off, w in ((0, 320), (320, 306)):
                          ph = psh.tile([128, 320], f32, tag="h")
                          for j, (ogi, part) in enumerate(srcs):
                              nc.tensor.matmul(ph[:, :w], w1t[:, ogi * 2 + og, :],
                                               part[:, ogi, off:off + w],
                                               start=(j == 0), stop=(j == len(srcs) - 1))
                          nc.scalar.activation(h[:, og, off:off + w], ph[:, :w], AF.Relu,
                                               scale=bnh[:, og:og + 1], bias=bnh[:, 2 + og:3 + og])

                  out_s = fp.tile([128, 640], f32, tag="outs")
                  for off, w in ((0, 320), (320, 306)):
                      po = psh.tile([128, 320], f32, tag="h")
                      nc.tensor.matmul(po[0:20, :w], w2t[:, 0, :], h[:, 0, off:off + w],
                                       start=True, stop=False)
                      nc.tensor.matmul(po[0:20, :w], w2t[:, 1, :], h[:, 1, off:off + w],
                                       start=False, stop=True)
                      nc.scalar.activation(out_s[0:20, off:off + w], po[0:20, :w],
                                           AF.Identity, bias=b2t[0:20, 0:1])
                  nc.sync.dma_start(out=out_d[s, :, :], in_=out_s[0:20, 0:625])

    nc.compile()
    return nc


def _pack(inputs):
    f32 = np.float32
    try:
        import ml_dtypes
        bf16 = ml_dtypes.bfloat16
    except ImportError:
        import jax.numpy as jnp
        bf16 = jnp.bfloat16
    kern = np.ascontiguousarray(inputs["kernel"], dtype=f32)
    search = np.ascontiguousarray(inputs["search"], dtype=f32)
    wk, ws = inputs["wk"].astype(f32), inputs["ws"].astype(f32)
    w1, w2, b2 = inputs["w1"].astype(f32), inputs["w2"].astype(f32), inputs["b2"].astype(f32)

    def fold(scale, bias, mean, var):
        inv = scale.astype(f32) / np.sqrt(var.astype(f32) + EPS)
        sh = bias.astype(f32) - mean.astype(f32) * inv
        arr = np.zeros((128, 4), f32)
        arr[:, 0:2] = inv.reshape(2, 128).T
        arr[:, 2:4] = sh.reshape(2, 128).T
        return arr

    bnk = fold(inputs["bnk_scale"], inputs["bnk_bias"], inputs["bnk_mean"], inputs["bnk_var"])
    bns = fold(inputs["bns_scale"], inputs["bns_bias"], inputs["bns_mean"], inputs["bns_var"])
    bnh = fold(inputs["bnh_scale"], inputs["bnh_bias"], inputs["bnh_mean"], inputs["bnh_var"])

    # conv weights -> lhsT tiles [ci, (cig,dy,dx,og), co]
    def conv_w(w):
        w6 = w.reshape(2, 128, 2, 128, 3, 3)           # og co cig ci dy dx
        return np.ascontiguousarray(
            w6.transpose(3, 2, 4, 5, 0, 1).reshape(128, 36, 128).astype(bf16))

    wkt, wst = conv_w(wk), conv_w(ws)
    w1t = np.ascontiguousarray(
        w1[:, :, 0, 0].reshape(2, 128, 2, 128).transpose(3, 2, 0, 1).reshape(128, 4, 128))
    w2t = np.ascontiguousarray(
        w2[:, :, 0, 0].reshape(20, 2, 128).transpose(2, 1, 0))
    b2t = np.zeros((128, 1), f32)
    b2t[:20, 0] = b2
    ident = np.eye(128, dtype=f32)

    # search [64,256,31,31] -> per core [8, 128(ci), 2(cig), 961]
    sr = np.zeros((NCORES, BPC, 128, 2, 968), bf16)
    sr[..., :961] = search.reshape(NCORES, BPC, 2, 128, 961).transpose(0, 1, 3, 2, 4).astype(bf16)
    # kernel [64,256,7,7] -> per core [128(ci), 2(cig), 8(s), 49]
    kr = np.zeros((NCORES, 128, 2, BPC, 52), bf16)
    kr[..., :49] = kern.reshape(NCORES, BPC, 2, 128, 49).transpose(0, 3, 2, 1, 4).astype(bf16)

    in_maps = []
    for c in range(NCORES):
        in_maps.append({
            "search": np.ascontiguousarray(sr[c]),
            "tmpl": np.ascontiguousarray(kr[c]),
            "wkt": wkt, "wst": wst, "w1t": w1t, "w2t": w2t,
            "bnk": bnk, "bns": bns, "bnh": bnh, "b2t": b2t, "ident": ident,
        })
    return in_maps


def get_program(bench_R=0):
    key = f"nc{bench_R}"
    if key not in _CACHE:
        _CACHE[key] = _build(bench_R)
    return _CACHE[key]


def kernel(**inputs):
    from concourse.bass_utils import run_bass_kernel_spmd
    nc = get_program()
    in_maps = _pack(inputs)
    res = run_bass_kernel_spmd(nc, in_maps, core_ids=list(range(NCORES)))
    out = np.stack([res.results[c]["out"] for c in range(NCORES)], axis=0)
    return out.reshape(64, 20, 25, 25).astype(np.float32)


# revision 28
# speedup vs baseline: 1.2360x; 1.0100x over previous
"""Trainium2 Bass kernel for nn_DepthwiseXCorr (SiamRPN++-style depthwise-xcorr head).

Pipeline per sample (data-parallel over batch: 64 samples -> 8 cores x 8):
  conv3x3(kernel,wk)+BN+ReLU -> k_feat [256,5,5]
  conv3x3(search,ws)+BN+ReLU -> s_feat [256,29,29]
  depthwise xcorr(s_feat,k_feat) -> feat [256,25,25]
  1x1 conv w1 + BN + ReLU -> h [256,25,25]
  1x1 conv w2 + b2 -> out [20,25,25]

Convolutions run on the PE in bf16 (inputs/weights quantized host-side; the
accumulation stays fp32 in PSUM). The depthwise xcorr is spread over four
engines: a few taps on the PE (per-tap diagonal-weight matmuls into PSUM),
the Activation engine bridges the PSUM partial into SBUF, then the Pool and
DVE engines chain scalar_tensor_tensor multiply-accumulates in place.
"""
import numpy as np

EPS = 1e-5
NCORES = 8
BPC = 8          # samples per core

# per-(sample, og) tap split: (n_pe, n_act_pool, n_dve) summing to 25.
# n_pe taps run on the PE (diag matmuls, diags made in batches of 5 on DVE,
# so n_pe must be a multiple of 5); n_act_pool taps are computed as products
# on the Activation engine and accumulated by Pool tensor_tensor adds; the
# rest chain scalar_tensor_tensor MACs on the DVE.
SCHED = [[(4, 7, 14), (9, 8, 8)] for _ in range(BPC - 1)] + [[(20, 0, 5), (24, 0, 1)]]
CK200 = True      # conv_kernel via 4D AP at N=200 (else N=288 windows)
DMAQ = 0          # 0: all weights on sync; 1: spread across queues
MERGE_B = 2       # 2: merge partial B into featc with a DVE add (head reads 1 partial)
WSPLIT = True     # split wkt/wst DMAs across sync+gpsimd queues
SEED_PSUM = True  # DVE chain seeds by reading px from PSUM (no Act bridges)
DIAG_ACT = 1      # og < DIAG_ACT gets per-diag creation on Act instead of DVE batch
PSC, PSX, PSH = 2, 2, 4
SPB, FPB, DPB, MPB = 3, 3, 6, 8
SINQ = 0          # 0: s_in DMA on sync queue; 1: on scalar (Act) queue

_CACHE = {}


def _shift_window(ap_2d, base_off, rows, cols, rowstride):
    """AP reading [128, rows, cols] window at element offset base_off of a
    [128, W] SBUF view, row stride in elements."""
    import concourse.bass as bass
    return bass.AP(
        tensor=ap_2d.tensor,
        offset=ap_2d.offset + base_off,
        ap=[list(ap_2d.ap[0]), [rowstride, rows], [1, cols]],
    )


def _build(bench_R=0):
    import concourse.bacc as bacc
    import concourse.bass as bass
    import concourse.mybir as mybir
    import concourse.tile as tile

    f32 = mybir.dt.float32
    f32r = mybir.dt.float32r
    bf16 = mybir.dt.bfloat16
    AF = mybir.ActivationFunctionType
    ALU = mybir.AluOpType

    nc = bacc.Bacc("TRN2", target_bir_lowering=False, debug=False,
                   num_devices=NCORES)

    search_d = nc.declare_dram_parameter("search", [BPC, 128, 2, 968], bf16, isOutput=False)
    tmpl_d = nc.declare_dram_parameter("tmpl", [128, 2, BPC, 52], bf16, isOutput=False)
    wkt_d = nc.declare_dram_parameter("wkt", [128, 36, 128], bf16, isOutput=False)
    wst_d = nc.declare_dram_parameter("wst", [128, 36, 128], bf16, isOutput=False)
    w1t_d = nc.declare_dram_parameter("w1t", [128, 4, 128], f32r, isOutput=False)
    w2t_d = nc.declare_dram_parameter("w2t", [128, 2, 20], f32r, isOutput=False)
    bnk_d = nc.declare_dram_parameter("bnk", [128, 4], f32, isOutput=False)
    bns_d = nc.declare_dram_parameter("bns", [128, 4], f32, isOutput=False)
    bnh_d = nc.declare_dram_parameter("bnh", [128, 4], f32, isOutput=False)
    b2_d = nc.declare_dram_parameter("b2t", [128, 1], f32, isOutput=False)
    id_d = nc.declare_dram_parameter("ident", [128, 128], f32, isOutput=False)
    out_d = nc.declare_dram_parameter("out", [BPC, 20, 625], f32, isOutput=True)

    def tidx(cig, dy, dx, og):
        return ((cig * 3 + dy) * 3 + dx) * 2 + og

    with tile.TileContext(nc) as tc:
        with (
            tc.tile_pool(name="wp", bufs=1) as wp,
            tc.tile_pool(name="sp", bufs=SPB) as sp,
            tc.tile_pool(name="fp", bufs=FPB) as fp,
            tc.tile_pool(name="dp", bufs=DPB) as dp,
            tc.tile_pool(name="mp", bufs=MPB) as mp,
            tc.tile_pool(name="psc", bufs=PSC, space="PSUM") as psc,
            tc.tile_pool(name="psx", bufs=PSX, space="PSUM") as psx,
            tc.tile_pool(name="psh", bufs=PSH, space="PSUM") as psh,
        ):
            wkt = wp.tile([128, 36, 128], bf16)
            wst = wp.tile([128, 36, 128], bf16)
            w1t = wp.tile([128, 4, 128], f32r)
            w2t = wp.tile([128, 2, 20], f32r)
            bnk = wp.tile([128, 4], f32)
            bns = wp.tile([128, 4], f32)
            bnh = wp.tile([128, 4], f32)
            b2t = wp.tile([128, 1], f32)
            ident = wp.tile([128, 128], f32)
            k_in = wp.tile([128, 2, BPC, 52], bf16)
            nc.gpsimd.dma_start(out=k_in, in_=tmpl_d[:, :, :, :])
            nc.gpsimd.dma_start(out=bnk, in_=bnk_d[:, :])
            if WSPLIT:
                nc.sync.dma_start(out=wkt[:, 0:18, :], in_=wkt_d[:, 0:18, :])
                nc.gpsimd.dma_start(out=wkt[:, 18:36, :], in_=wkt_d[:, 18:36, :])
                nc.sync.dma_start(out=wst[:, 0:18, :], in_=wst_d[:, 0:18, :])
                nc.gpsimd.dma_start(out=wst[:, 18:36, :], in_=wst_d[:, 18:36, :])
            elif DMAQ == 1:
                nc.gpsimd.dma_start(out=wkt, in_=wkt_d[:, :, :])
                nc.scalar.dma_start(out=wst, in_=wst_d[:, :, :])
            else:
                nc.sync.dma_start(out=wkt, in_=wkt_d[:, :, :])
                nc.sync.dma_start(out=wst, in_=wst_d[:, :, :])
            nc.sync.dma_start(out=w1t, in_=w1t_d[:, :, :])
            nc.sync.dma_start(out=w2t, in_=w2t_d[:, :, :])
            nc.gpsimd.dma_start(out=bns, in_=bns_d[:, :])
            nc.gpsimd.dma_start(out=bnh, in_=bnh_d[:, :])
            nc.gpsimd.dma_start(out=b2t, in_=b2_d[:, :])
            nc.gpsimd.dma_start(out=ident, in_=id_d[:, :])

            # conv_kernel branch: all samples batched
            kfs = 25 if CK200 else 36
            k_feat = wp.tile([128, 2, BPC * kfs], f32)
            for og in range(2):
                pk = psc.tile([128, 512], f32, tag="conv")
                j = 0
                for cig in range(2):
                    for dy in range(3):
                        for dx in range(3):
                            base = k_in[:, cig, :, :]
                            if CK200:
                                rhs = bass.AP(tensor=base.tensor,
                                              offset=base.offset + dy * 7 + dx,
                                              ap=[list(base.ap[0]), [52, BPC], [7, 5], [1, 5]])
                            else:
                                rhs = base[:, :, dy * 7 + dx: dy * 7 + dx + 36]
                            nc.tensor.matmul(pk[:, :BPC * kfs], wkt[:, tidx(cig, dy, dx, og), :],
                                             rhs, start=(j == 0), stop=(j == 17))
                            j += 1
                nc.scalar.activation(k_feat[:, og, :], pk[:, :BPC * kfs], AF.Relu,
                                     scale=bnk[:, og:og + 1], bias=bnk[:, 2 + og:3 + og])

            taps = [(t // 5, t % 5) for t in range(25)]

            import contextlib
            loop_cm = (tc.For_i(0, bench_R, 1,
                                  hint_engines=(mybir.EngineType.PE,
                                                mybir.EngineType.DVE,
                                                mybir.EngineType.Activation))
                         if bench_R else contextlib.nullcontext())
            with loop_cm:
              for s in range(BPC):
                  s_in = sp.tile([128, 2, 968], bf16, tag="s_in")
                  if SINQ:
                      nc.scalar.dma_start(out=s_in, in_=search_d[s, :, :, :])
                  else:
                      nc.sync.dma_start(out=s_in, in_=search_d[s, :, :, :])

                  # conv_search: out plane 29 rows x 29 cols, packed stride 29
                  s_feat = sp.tile([128, 2, 841], f32r, tag="s_feat")
                  for og in range(2):
                      for off, y0c, rws in ((0, 0, 17), (493, 17, 12)):
                          w = rws * 29
                          pc = psc.tile([128, 512], f32, tag="conv")
                          j = 0
                          for cig in range(2):
                              for dy in range(3):
                                  for dx in range(3):
                                      rhs = _shift_window(s_in[:, cig, :], (y0c + dy) * 31 + dx,
                                                          rws, 29, 31)
                                      nc.tensor.matmul(pc[:, :w], wst[:, tidx(cig, dy, dx, og), :],
                                                       rhs, start=(j == 0), stop=(j == 17))
                                      j += 1
                          nc.scalar.activation(s_feat[:, og, off:off + w], pc[:, :w], AF.Relu,
                                               scale=bns[:, og:og + 1], bias=bns[:, 2 + og:3 + og])

                  # depthwise xcorr, two independent partials per og:
                  #   A: PE diag taps -> PSUM -> Act bridge -> DVE stt chain -> featr (f32r)
                  #   B: Act tap products, Pool tensor_tensor add chain -> fpr (f32r)
                  # the head matmul accumulates both partials in PSUM.
                  featc = fp.tile([128, 2, 625], f32, tag="featc")
                  featr = fp.tile([128, 2, 640], f32r, tag="featr")
                  fpool = fp.tile([128, 2, 625], f32, tag="fpool")
                  fpr = fp.tile([128, 2, 640], f32r, tag="fpr")
                  nc.gpsimd.memset(featr[:, :, 625:640].bitcast(f32), 0.0)
                  nc.gpsimd.memset(fpr[:, :, 625:640].bitcast(f32), 0.0)
                  for og in range(2):
                      npe, nap, ndve = SCHED[s][og]
                      assert 1 <= npe <= 24
                      pe_taps = taps[:npe]
                      act_taps = taps[npe:npe + nap]
                      dve_taps = taps[npe + nap:]
                      sf = s_feat[:, og, :]
                      kf = k_feat[:, og, :]
                      krs2 = 5 if CK200 else 7
                      kbase = s * kfs

                      # diag batches on DVE: dg[:, i, :] = ident * k[tap i]
                      # (or per-diag Act creation when og < DIAG_ACT)
                      krs = 5 if CK200 else 7
                      dlist = []
                      if og < DIAG_ACT:
                          for (dy, dx) in pe_taps:
                              dga = dp.tile([128, 128], f32r, tag="diaga")
                              nc.scalar.activation(
                                  dga, ident, AF.Copy,
                                  scale=kf[:, kbase + dy * krs + dx: kbase + dy * krs + dx + 1])
                              dlist.append(dga)
                      else:
                          done = 0
                          for dy in range((npe + 4) // 5):
                              nb = min(5, npe - done)
                              dg = dp.tile([128, 5, 128], f32r, tag="diag")
                              id_b = bass.AP(tensor=ident[:, :].tensor, offset=ident[:, :].offset,
                                             ap=[list(ident[:, :].ap[0]), [0, nb], [1, 128]])
                              k_b = bass.AP(tensor=kf.tensor, offset=kf.offset + kbase + dy * krs,
                                            ap=[list(kf.ap[0]), [1, nb], [0, 128]])
                              nc.vector.tensor_tensor(dg[:, 0:nb, :], id_b, k_b, ALU.mult)
                              for i in range(nb):
                                  dlist.append(dg[:, i, :])
                              done += nb

                      # PE partial: diag-weight matmuls accumulated in PSUM.
                      # fp32r matmul needs even innermost counts -> 26-wide
                      # windows; tap (4,4) would read past s_feat so PE taps
                      # must come from the row-major prefix (dy<4).
                      assert all((dy, dx) != (4, 4) for dy, dx in pe_taps)
                      pxs = []
                      for y0, rows in ((0, 13), (13, 12)):
                          n = rows * 26
                          px = psx.tile([128, 338], f32, tag="x")
                          for i, (dy, dx) in enumerate(pe_taps):
                              rhs = _shift_window(sf, (y0 + dy) * 29 + dx, rows, 26, 29)
                              nc.tensor.matmul(px[:, :n], dlist[i], rhs,
                                               start=(i == 0), stop=(i == npe - 1))
                          pxs.append((y0, rows, px))
                          if not SEED_PSUM:
                              # bridge PSUM partial into SBUF (Act engine)
                              src_px = _shift_window(px, 0, rows, 25, 26)
                              dst_f = featc[:, og, y0 * 25: y0 * 25 + rows * 25].rearrange(
                                  "p (r c) -> p r c", c=25)
                              nc.scalar.activation(dst_f, src_px, AF.Copy)

                      fv = featc[:, og, :].rearrange("p (r c) -> p r c", c=25)
                      frv = featr[:, og, 0:625].rearrange("p (r c) -> p r c", c=25)
                      gv = fpool[:, og, :].rearrange("p (r c) -> p r c", c=25)
                      grv = fpr[:, og, 0:625].rearrange("p (r c) -> p r c", c=25)
                      # partial B: Act products, Pool add chain; without
                      # MERGE_B the last add writes the f32r tile fpr (second
                      # head partial), with MERGE_B it stays in fpool and a
                      # final Pool add folds it into featc before the last
                      # DVE tap.
                      assert len(act_taps) == 0 or len(act_taps) >= 2
                      for j, (dy, dx) in enumerate(act_taps):
                          win = _shift_window(sf, dy * 29 + dx, 25, 25, 29)
                          kap = kf[:, kbase + dy * krs2 + dx: kbase + dy * krs2 + dx + 1]
                          if j == 0:
                              nc.scalar.activation(gv, win, AF.Copy, scale=kap)
                          else:
                              m = mp.tile([128, 25, 25], f32, tag="m")
                              nc.scalar.activation(m, win, AF.Copy, scale=kap)
                              dst = gv if MERGE_B or j < len(act_taps) - 1 else grv
                              nc.gpsimd.tensor_tensor(dst, gv, m, ALU.add)
                      # partial A: DVE chain; last tap writes the f32r tile
                      assert len(dve_taps) >= 1
                      for j, (dy, dx) in enumerate(dve_taps):
                          if MERGE_B and act_taps and j == len(dve_taps) - 1:
                              if MERGE_B == 2:
                                  nc.vector.tensor_tensor(fv, fv, gv, ALU.add)
                              else:
                                  nc.gpsimd.tensor_tensor(fv, fv, gv, ALU.add)
                          kap = kf[:, kbase + dy * krs2 + dx: kbase + dy * krs2 + dx + 1]
                          if SEED_PSUM and j == 0:
                              # seed: two stts, each adding a px PSUM slice; if
                              # this is also the last tap, write featr directly
                              # (requires no B partial to merge)
                              last = j == len(dve_taps) - 1
                              assert not (last and act_taps)
                              base = featr if last else featc
                              for y0, rows, px in pxs:
                                  winp = _shift_window(sf, (dy + y0) * 29 + dx, rows, 25, 29)
                                  pxv = _shift_window(px, 0, rows, 25, 26)
                                  dstp = base[:, og, y0 * 25: y0 * 25 + rows * 25].rearrange(
                                      "p (r c) -> p r c", c=25)
                                  nc.vector.scalar_tensor_tensor(dstp, winp, kap, pxv,
                                                                 ALU.mult, ALU.add)
                              continue
                          win = _shift_window(sf, dy * 29 + dx, 25, 25, 29)
                          dst = frv if j == len(dve_taps) - 1 else fv
                          nc.vector.scalar_tensor_tensor(dst, win, kap, fv, ALU.mult, ALU.add)

                  # head: 1x1 conv -> BN -> ReLU -> 1x1 conv + b2
                  # (accumulates the live xcorr partials per input og)
                  h = fp.tile([128, 2, 640], f32r, tag="h")
                  for og in range(2):
                      srcs = []
                      for ogi in range(2):
                          srcs.append((ogi, featr))
                          if SCHED[s][ogi][1] > 0 and not MERGE_B:
                              srcs.append((ogi, fpr))
                      for off, w in ((0, 320), (320, 306)):
                          ph = psh.tile([128, 320], f32, tag="h")
                          for j, (ogi, part) in enumerate(srcs):
                              nc.tensor.matmul(ph[:, :w], w1t[:, ogi * 2 + og, :],
                                               part[:, ogi, off:off + w],
                                               start=(j == 0), stop=(j == len(srcs) - 1))
                          nc.scalar.activation(h[:, og, off:off + w], ph[:, :w], AF.Relu,
                                               scale=bnh[:, og:og + 1], bias=bnh[:, 2 + og:3 + og])

                  out_s = fp.tile([128, 640], f32, tag="outs")
                  for off, w in ((0, 320), (320, 306)):
                      po = psh.tile([128, 320], f32, tag="h")
                      nc.tensor.matmul(po[0:20, :w], w2t[:, 0, :], h[:, 0, off:off + w],
                                       start=True, stop=False)
                      nc.tensor.matmul(po[0:20, :w], w2t[:, 1, :], h[:, 1, off:off + w],
                                       start=False, stop=True)
                      nc.scalar.activation(out_s[0:20, off:off + w], po[0:20, :w],
                                           AF.Identity, bias=b2t[0:20, 0:1])
                  nc.sync.dma_start(out=out_d[s, :, :], in_=out_s[0:20, 0:625])

    nc.compile()
    return nc


def _pack(inputs):
    f32 = np.float32
    try:
        import ml_dtypes
        bf16 = ml_dtypes.bfloat16
    except ImportError:
        import jax.numpy as jnp
        bf16 = jnp.bfloat16
    kern = np.ascontiguousarray(inputs["kernel"], dtype=f32)
    search = np.ascontiguousarray(inputs["search"], dtype=f32)
    wk, ws = inputs["wk"].astype(f32), inputs["ws"].astype(f32)
    w1, w2, b2 = inputs["w1"].astype(f32), inputs["w2"].astype(f32), inputs["b2"].astype(f32)

    def fold(scale, bias, mean, var):
        inv = scale.astype(f32) / np.sqrt(var.astype(f32) + EPS)
        sh = bias.astype(f32) - mean.astype(f32) * inv
        arr = np.zeros((128, 4), f32)
        arr[:, 0:2] = inv.reshape(2, 128).T
        arr[:, 2:4] = sh.reshape(2, 128).T
        return arr

    bnk = fold(inputs["bnk_scale"], inputs["bnk_bias"], inputs["bnk_mean"], inputs["bnk_var"])
    bns = fold(inputs["bns_scale"], inputs["bns_bias"], inputs["bns_mean"], inputs["bns_var"])
    bnh = fold(inputs["bnh_scale"], inputs["bnh_bias"], inputs["bnh_mean"], inputs["bnh_var"])

    # conv weights -> lhsT tiles [ci, (cig,dy,dx,og), co]
    def conv_w(w):
        w6 = w.reshape(2, 128, 2, 128, 3, 3)           # og co cig ci dy dx
        return np.ascontiguousarray(
            w6.transpose(3, 2, 4, 5, 0, 1).reshape(128, 36, 128).astype(bf16))

    wkt, wst = conv_w(wk), conv_w(ws)
    w1t = np.ascontiguousarray(
        w1[:, :, 0, 0].reshape(2, 128, 2, 128).transpose(3, 2, 0, 1).reshape(128, 4, 128))
    w2t = np.ascontiguousarray(
        w2[:, :, 0, 0].reshape(20, 2, 128).transpose(2, 1, 0))
    b2t = np.zeros((128, 1), f32)
    b2t[:20, 0] = b2
    ident = np.eye(128, dtype=f32)

    # search [64,256,31,31] -> per core [8, 128(ci), 2(cig), 961]
    sr = np.zeros((NCORES, BPC, 128, 2, 968), bf16)
    sr[..., :961] = search.reshape(NCORES, BPC, 2, 128, 961).transpose(0, 1, 3, 2, 4).astype(bf16)
    # kernel [64,256,7,7] -> per core [128(ci), 2(cig), 8(s), 49]
    kr = np.zeros((NCORES, 128, 2, BPC, 52), bf16)
    kr[..., :49] = kern.reshape(NCORES, BPC, 2, 128, 49).transpose(0, 3, 2, 1, 4).astype(bf16)

    in_maps = []
    for c in range(NCORES):
        in_maps.append({
            "search": np.ascontiguousarray(sr[c]),
            "tmpl": np.ascontiguousarray(kr[c]),
            "wkt": wkt, "wst": wst, "w1t": w1t, "w2t": w2t,
            "bnk": bnk, "bns": bns, "bnh": bnh, "b2t": b2t, "ident": ident,
        })
    return in_maps


def get_program(bench_R=0):
    key = f"nc{bench_R}"
    if key not in _CACHE:
        _CACHE[key] = _build(bench_R)
    return _CACHE[key]


def kernel(**inputs):
    from concourse.bass_utils import run_bass_kernel_spmd
    nc = get_program()
    in_maps = _pack(inputs)
    res = run_bass_kernel_spmd(nc, in_maps, core_ids=list(range(NCORES)))
    out = np.stack([res.results[c]["out"] for c in range(NCORES)], axis=0)
    return out.reshape(64, 20, 25, 25).astype(np.float32)


# revision 29
# speedup vs baseline: 1.2379x; 1.0015x over previous
"""Trainium2 Bass kernel for nn_DepthwiseXCorr (SiamRPN++-style depthwise-xcorr head).

Pipeline per sample (data-parallel over batch: 64 samples -> 8 cores x 8):
  conv3x3(kernel,wk)+BN+ReLU -> k_feat [256,5,5]
  conv3x3(search,ws)+BN+ReLU -> s_feat [256,29,29]
  depthwise xcorr(s_feat,k_feat) -> feat [256,25,25]
  1x1 conv w1 + BN + ReLU -> h [256,25,25]
  1x1 conv w2 + b2 -> out [20,25,25]

Convolutions run on the PE in bf16 (inputs/weights quantized host-side; the
accumulation stays fp32 in PSUM). The depthwise xcorr is spread over four
engines: a few taps on the PE (per-tap diagonal-weight matmuls into PSUM),
the Activation engine bridges the PSUM partial into SBUF, then the Pool and
DVE engines chain scalar_tensor_tensor multiply-accumulates in place.
"""
import numpy as np

EPS = 1e-5
NCORES = 8
BPC = 8          # samples per core

# per-(sample, og) tap split: (n_pe, n_act_pool, n_dve) summing to 25.
# n_pe taps run on the PE (diag matmuls, diags made in batches of 5 on DVE,
# so n_pe must be a multiple of 5); n_act_pool taps are computed as products
# on the Activation engine and accumulated by Pool tensor_tensor adds; the
# rest chain scalar_tensor_tensor MACs on the DVE.
_SA = [(4, 7, 14), (9, 8, 8)]
_SB = [(5, 7, 13), (8, 8, 9)]
SCHED = [_SB, _SA, _SB, _SA, _SB, _SA, _SB, [(20, 0, 5), (24, 0, 1)]]
CK200 = True      # conv_kernel via 4D AP at N=200 (else N=288 windows)
DMAQ = 0          # 0: all weights on sync; 1: spread across queues
MERGE_B = 2       # 2: merge partial B into featc with a DVE add (head reads 1 partial)
WSPLIT = True     # split wkt/wst DMAs across sync+gpsimd queues
SEED_PSUM = True  # DVE chain seeds by reading px from PSUM (no Act bridges)
DIAG_ACT = 1      # og < DIAG_ACT gets per-diag creation on Act instead of DVE batch
PSC, PSX, PSH = 2, 2, 4
SPB, FPB, DPB, MPB = 3, 3, 6, 8
SINQ = 0          # 0: s_in DMA on sync queue; 1: on scalar (Act) queue

_CACHE = {}


def _shift_window(ap_2d, base_off, rows, cols, rowstride):
    """AP reading [128, rows, cols] window at element offset base_off of a
    [128, W] SBUF view, row stride in elements."""
    import concourse.bass as bass
    return bass.AP(
        tensor=ap_2d.tensor,
        offset=ap_2d.offset + base_off,
        ap=[list(ap_2d.ap[0]), [rowstride, rows], [1, cols]],
    )


def _build(bench_R=0):
    import concourse.bacc as bacc
    import concourse.bass as bass
    import concourse.mybir as mybir
    import concourse.tile as tile

    f32 = mybir.dt.float32
    f32r = mybir.dt.float32r
    bf16 = mybir.dt.bfloat16
    AF = mybir.ActivationFunctionType
    ALU = mybir.AluOpType

    nc = bacc.Bacc("TRN2", target_bir_lowering=False, debug=False,
                   num_devices=NCORES)

    search_d = nc.declare_dram_parameter("search", [BPC, 128, 2, 968], bf16, isOutput=False)
    tmpl_d = nc.declare_dram_parameter("tmpl", [128, 2, BPC, 52], bf16, isOutput=False)
    wkt_d = nc.declare_dram_parameter("wkt", [128, 36, 128], bf16, isOutput=False)
    wst_d = nc.declare_dram_parameter("wst", [128, 36, 128], bf16, isOutput=False)
    w1t_d = nc.declare_dram_parameter("w1t", [128, 4, 128], f32r, isOutput=False)
    w2t_d = nc.declare_dram_parameter("w2t", [128, 2, 20], f32r, isOutput=False)
    bnk_d = nc.declare_dram_parameter("bnk", [128, 4], f32, isOutput=False)
    bns_d = nc.declare_dram_parameter("bns", [128, 4], f32, isOutput=False)
    bnh_d = nc.declare_dram_parameter("bnh", [128, 4], f32, isOutput=False)
    b2_d = nc.declare_dram_parameter("b2t", [128, 1], f32, isOutput=False)
    id_d = nc.declare_dram_parameter("ident", [128, 128], f32, isOutput=False)
    out_d = nc.declare_dram_parameter("out", [BPC, 20, 625], f32, isOutput=True)

    def tidx(cig, dy, dx, og):
        return ((cig * 3 + dy) * 3 + dx) * 2 + og

    with tile.TileContext(nc) as tc:
        with (
            tc.tile_pool(name="wp", bufs=1) as wp,
            tc.tile_pool(name="sp", bufs=SPB) as sp,
            tc.tile_pool(name="fp", bufs=FPB) as fp,
            tc.tile_pool(name="dp", bufs=DPB) as dp,
            tc.tile_pool(name="mp", bufs=MPB) as mp,
            tc.tile_pool(name="psc", bufs=PSC, space="PSUM") as psc,
            tc.tile_pool(name="psx", bufs=PSX, space="PSUM") as psx,
            tc.tile_pool(name="psh", bufs=PSH, space="PSUM") as psh,
        ):
            wkt = wp.tile([128, 36, 128], bf16)
            wst = wp.tile([128, 36, 128], bf16)
            w1t = wp.tile([128, 4, 128], f32r)
            w2t = wp.tile([128, 2, 20], f32r)
            bnk = wp.tile([128, 4], f32)
            bns = wp.tile([128, 4], f32)
            bnh = wp.tile([128, 4], f32)
            b2t = wp.tile([128, 1], f32)
            ident = wp.tile([128, 128], f32)
            k_in = wp.tile([128, 2, BPC, 52], bf16)
            nc.gpsimd.dma_start(out=k_in, in_=tmpl_d[:, :, :, :])
            nc.gpsimd.dma_start(out=bnk, in_=bnk_d[:, :])
            if WSPLIT:
                nc.sync.dma_start(out=wkt[:, 0:18, :], in_=wkt_d[:, 0:18, :])
                nc.gpsimd.dma_start(out=wkt[:, 18:36, :], in_=wkt_d[:, 18:36, :])
                nc.sync.dma_start(out=wst[:, 0:18, :], in_=wst_d[:, 0:18, :])
                nc.gpsimd.dma_start(out=wst[:, 18:36, :], in_=wst_d[:, 18:36, :])
            elif DMAQ == 1:
                nc.gpsimd.dma_start(out=wkt, in_=wkt_d[:, :, :])
                nc.scalar.dma_start(out=wst, in_=wst_d[:, :, :])
            else:
                nc.sync.dma_start(out=wkt, in_=wkt_d[:, :, :])
                nc.sync.dma_start(out=wst, in_=wst_d[:, :, :])
            nc.sync.dma_start(out=w1t, in_=w1t_d[:, :, :])
            nc.sync.dma_start(out=w2t, in_=w2t_d[:, :, :])
            nc.gpsimd.dma_start(out=bns, in_=bns_d[:, :])
            nc.gpsimd.dma_start(out=bnh, in_=bnh_d[:, :])
            nc.gpsimd.dma_start(out=b2t, in_=b2_d[:, :])
            nc.gpsimd.dma_start(out=ident, in_=id_d[:, :])

            # conv_kernel branch: all samples batched
            kfs = 25 if CK200 else 36
            k_feat = wp.tile([128, 2, BPC * kfs], f32)
            for og in range(2):
                pk = psc.tile([128, 512], f32, tag="conv")
                j = 0
                for cig in range(2):
                    for dy in range(3):
                        for dx in range(3):
                            base = k_in[:, cig, :, :]
                            if CK200:
                                rhs = bass.AP(tensor=base.tensor,
                                              offset=base.offset + dy * 7 + dx,
                                              ap=[list(base.ap[0]), [52, BPC], [7, 5], [1, 5]])
                            else:
                                rhs = base[:, :, dy * 7 + dx: dy * 7 + dx + 36]
                            nc.tensor.matmul(pk[:, :BPC * kfs], wkt[:, tidx(cig, dy, dx, og), :],
                                             rhs, start=(j == 0), stop=(j == 17))
                            j += 1
                nc.scalar.activation(k_feat[:, og, :], pk[:, :BPC * kfs], AF.Relu,
                                     scale=bnk[:, og:og + 1], bias=bnk[:, 2 + og:3 + og])

            taps = [(t // 5, t % 5) for t in range(25)]

            import contextlib
            loop_cm = (tc.For_i(0, bench_R, 1,
                                  hint_engines=(mybir.EngineType.PE,
                                                mybir.EngineType.DVE,
                                                mybir.EngineType.Activation))
                         if bench_R else contextlib.nullcontext())
            with loop_cm:
              for s in range(BPC):
                  s_in = sp.tile([128, 2, 968], bf16, tag="s_in")
                  if SINQ:
                      nc.scalar.dma_start(out=s_in, in_=search_d[s, :, :, :])
                  else:
                      nc.sync.dma_start(out=s_in, in_=search_d[s, :, :, :])

                  # conv_search: out plane 29 rows x 29 cols, packed stride 29
                  s_feat = sp.tile([128, 2, 841], f32r, tag="s_feat")
                  for og in range(2):
                      for off, y0c, rws in ((0, 0, 17), (493, 17, 12)):
                          w = rws * 29
                          pc = psc.tile([128, 512], f32, tag="conv")
                          j = 0
                          for cig in range(2):
                              for dy in range(3):
                                  for dx in range(3):
                                      rhs = _shift_window(s_in[:, cig, :], (y0c + dy) * 31 + dx,
                                                          rws, 29, 31)
                                      nc.tensor.matmul(pc[:, :w], wst[:, tidx(cig, dy, dx, og), :],
                                                       rhs, start=(j == 0), stop=(j == 17))
                                      j += 1
                          nc.scalar.activation(s_feat[:, og, off:off + w], pc[:, :w], AF.Relu,
                                               scale=bns[:, og:og + 1], bias=bns[:, 2 + og:3 + og])

                  # depthwise xcorr, two independent partials per og:
                  #   A: PE diag taps -> PSUM -> Act bridge -> DVE stt chain -> featr (f32r)
                  #   B: Act tap products, Pool tensor_tensor add chain -> fpr (f32r)
                  # the head matmul accumulates both partials in PSUM.
                  featc = fp.tile([128, 2, 625], f32, tag="featc")
                  featr = fp.tile([128, 2, 640], f32r, tag="featr")
                  fpool = fp.tile([128, 2, 625], f32, tag="fpool")
                  fpr = fp.tile([128, 2, 640], f32r, tag="fpr")
                  nc.gpsimd.memset(featr[:, :, 625:640].bitcast(f32), 0.0)
                  nc.gpsimd.memset(fpr[:, :, 625:640].bitcast(f32), 0.0)
                  for og in range(2):
                      npe, nap, ndve = SCHED[s][og]
                      assert 1 <= npe <= 24
                      pe_taps = taps[:npe]
                      act_taps = taps[npe:npe + nap]
                      dve_taps = taps[npe + nap:]
                      sf = s_feat[:, og, :]
                      kf = k_feat[:, og, :]
                      krs2 = 5 if CK200 else 7
                      kbase = s * kfs

                      # diag batches on DVE: dg[:, i, :] = ident * k[tap i]
                      # (or per-diag Act creation when og < DIAG_ACT)
                      krs = 5 if CK200 else 7
                      dlist = []
                      if og < DIAG_ACT:
                          for (dy, dx) in pe_taps:
                              dga = dp.tile([128, 128], f32r, tag="diaga")
                              nc.scalar.activation(
                                  dga, ident, AF.Copy,
                                  scale=kf[:, kbase + dy * krs + dx: kbase + dy * krs + dx + 1])
                              dlist.append(dga)
                      else:
                          done = 0
                          for dy in range((npe + 4) // 5):
                              nb = min(5, npe - done)
                              dg = dp.tile([128, 5, 128], f32r, tag="diag")
                              id_b = bass.AP(tensor=ident[:, :].tensor, offset=ident[:, :].offset,
                                             ap=[list(ident[:, :].ap[0]), [0, nb], [1, 128]])
                              k_b = bass.AP(tensor=kf.tensor, offset=kf.offset + kbase + dy * krs,
                                            ap=[list(kf.ap[0]), [1, nb], [0, 128]])
                              nc.vector.tensor_tensor(dg[:, 0:nb, :], id_b, k_b, ALU.mult)
                              for i in range(nb):
                                  dlist.append(dg[:, i, :])
                              done += nb

                      # PE partial: diag-weight matmuls accumulated in PSUM.
                      # fp32r matmul needs even innermost counts -> 26-wide
                      # windows; tap (4,4) would read past s_feat so PE taps
                      # must come from the row-major prefix (dy<4).
                      assert all((dy, dx) != (4, 4) for dy, dx in pe_taps)
                      pxs = []
                      for y0, rows in ((0, 13), (13, 12)):
                          n = rows * 26
                          px = psx.tile([128, 338], f32, tag="x")
                          for i, (dy, dx) in enumerate(pe_taps):
                              rhs = _shift_window(sf, (y0 + dy) * 29 + dx, rows, 26, 29)
                              nc.tensor.matmul(px[:, :n], dlist[i], rhs,
                                               start=(i == 0), stop=(i == npe - 1))
                          pxs.append((y0, rows, px))
                          if not SEED_PSUM:
                              # bridge PSUM partial into SBUF (Act engine)
                              src_px = _shift_window(px, 0, rows, 25, 26)
                              dst_f = featc[:, og, y0 * 25: y0 * 25 + rows * 25].rearrange(
                                  "p (r c) -> p r c", c=25)
                              nc.scalar.activation(dst_f, src_px, AF.Copy)

                      fv = featc[:, og, :].rearrange("p (r c) -> p r c", c=25)
                      frv = featr[:, og, 0:625].rearrange("p (r c) -> p r c", c=25)
                      gv = fpool[:, og, :].rearrange("p (r c) -> p r c", c=25)
                      grv = fpr[:, og, 0:625].rearrange("p (r c) -> p r c", c=25)
                      # partial B: Act products, Pool add chain; without
                      # MERGE_B the last add writes the f32r tile fpr (second
                      # head partial), with MERGE_B it stays in fpool and a
                      # final Pool add folds it into featc before the last
                      # DVE tap.
                      assert len(act_taps) == 0 or len(act_taps) >= 2
                      for j, (dy, dx) in enumerate(act_taps):
                          win = _shift_window(sf, dy * 29 + dx, 25, 25, 29)
                          kap = kf[:, kbase + dy * krs2 + dx: kbase + dy * krs2 + dx + 1]
                          if j == 0:
                              nc.scalar.activation(gv, win, AF.Copy, scale=kap)
                          else:
                              m = mp.tile([128, 25, 25], f32, tag="m")
                              nc.scalar.activation(m, win, AF.Copy, scale=kap)
                              dst = gv if MERGE_B or j < len(act_taps) - 1 else grv
                              nc.gpsimd.tensor_tensor(dst, gv, m, ALU.add)
                      # partial A: DVE chain; last tap writes the f32r tile
                      assert len(dve_taps) >= 1
                      for j, (dy, dx) in enumerate(dve_taps):
                          if MERGE_B and act_taps and j == len(dve_taps) - 1:
                              if MERGE_B == 2:
                                  nc.vector.tensor_tensor(fv, fv, gv, ALU.add)
                              else:
                                  nc.gpsimd.tensor_tensor(fv, fv, gv, ALU.add)
                          kap = kf[:, kbase + dy * krs2 + dx: kbase + dy * krs2 + dx + 1]
                          if SEED_PSUM and j == 0:
                              # seed: two stts, each adding a px PSUM slice; if
                              # this is also the last tap, write featr directly
                              # (requires no B partial to merge)
                              last = j == len(dve_taps) - 1
                              assert not (last and act_taps)
                              base = featr if last else featc
                              for y0, rows, px in pxs:
                                  winp = _shift_window(sf, (dy + y0) * 29 + dx, rows, 25, 29)
                                  pxv = _shift_window(px, 0, rows, 25, 26)
                                  dstp = base[:, og, y0 * 25: y0 * 25 + rows * 25].rearrange(
                                      "p (r c) -> p r c", c=25)
                                  nc.vector.scalar_tensor_tensor(dstp, winp, kap, pxv,
                                                                 ALU.mult, ALU.add)
                              continue
                          win = _shift_window(sf, dy * 29 + dx, 25, 25, 29)
                          dst = frv if j == len(dve_taps) - 1 else fv
                          nc.vector.scalar_tensor_tensor(dst, win, kap, fv, ALU.mult, ALU.add)

                  # head: 1x1 conv -> BN -> ReLU -> 1x1 conv + b2
                  # (accumulates the live xcorr partials per input og)
                  h = fp.tile([128, 2, 640], f32r, tag="h")
                  for og in range(2):
                      srcs = []
                      for ogi in range(2):
                          srcs.append((ogi, featr))
                          if SCHED[s][ogi][1] > 0 and not MERGE_B:
                              srcs.append((ogi, fpr))
                      for off, w in ((0, 320), (320, 306)):
                          ph = psh.tile([128, 320], f32, tag="h")
                          for j, (ogi, part) in enumerate(srcs):
                              nc.tensor.matmul(ph[:, :w], w1t[:, ogi * 2 + og, :],
                                               part[:, ogi, off:off + w],
                                               start=(j == 0), stop=(j == len(srcs) - 1))
                          nc.scalar.activation(h[:, og, off:off + w], ph[:, :w], AF.Relu,
                                               scale=bnh[:, og:og + 1], bias=bnh[:, 2 + og:3 + og])

                  out_s = fp.tile([128, 640], f32, tag="outs")
                  for off, w in ((0, 320), (320, 306)):
                      po = psh.tile([128, 320], f32, tag="h")
                      nc.tensor.matmul(po[0:20, :w], w2t[:, 0, :], h[:, 0, off:off + w],
                                       start=True, stop=False)
                      nc.tensor.matmul(po[0:20, :w], w2t[:, 1, :], h[:, 1, off:off + w],
                                       start=False, stop=True)
                      nc.scalar.activation(out_s[0:20, off:off + w], po[0:20, :w],
                                           AF.Identity, bias=b2t[0:20, 0:1])
                  nc.sync.dma_start(out=out_d[s, :, :], in_=out_s[0:20, 0:625])

    nc.compile()
    return nc


def _pack(inputs):
    f32 = np.float32
    try:
        import ml_dtypes
        bf16 = ml_dtypes.bfloat16
    except ImportError:
        import jax.numpy as jnp
        bf16 = jnp.bfloat16
    kern = np.ascontiguousarray(inputs["kernel"], dtype=f32)
    search = np.ascontiguousarray(inputs["search"], dtype=f32)
    wk, ws = inputs["wk"].astype(f32), inputs["ws"].astype(f32)
    w1, w2, b2 = inputs["w1"].astype(f32), inputs["w2"].astype(f32), inputs["b2"].astype(f32)

    def fold(scale, bias, mean, var):
        inv = scale.astype(f32) / np.sqrt(var.astype(f32) + EPS)
        sh = bias.astype(f32) - mean.astype(f32) * inv
        arr = np.zeros((128, 4), f32)
        arr[:, 0:2] = inv.reshape(2, 128).T
        arr[:, 2:4] = sh.reshape(2, 128).T
        return arr

    bnk = fold(inputs["bnk_scale"], inputs["bnk_bias"], inputs["bnk_mean"], inputs["bnk_var"])
    bns = fold(inputs["bns_scale"], inputs["bns_bias"], inputs["bns_mean"], inputs["bns_var"])
    bnh = fold(inputs["bnh_scale"], inputs["bnh_bias"], inputs["bnh_mean"], inputs["bnh_var"])

    # conv weights -> lhsT tiles [ci, (cig,dy,dx,og), co]
    def conv_w(w):
        w6 = w.reshape(2, 128, 2, 128, 3, 3)           # og co cig ci dy dx
        return np.ascontiguousarray(
            w6.transpose(3, 2, 4, 5, 0, 1).reshape(128, 36, 128).astype(bf16))

    wkt, wst = conv_w(wk), conv_w(ws)
    w1t = np.ascontiguousarray(
        w1[:, :, 0, 0].reshape(2, 128, 2, 128).transpose(3, 2, 0, 1).reshape(128, 4, 128))
    w2t = np.ascontiguousarray(
        w2[:, :, 0, 0].reshape(20, 2, 128).transpose(2, 1, 0))
    b2t = np.zeros((128, 1), f32)
    b2t[:20, 0] = b2
    ident = np.eye(128, dtype=f32)

    # search [64,256,31,31] -> per core [8, 128(ci), 2(cig), 961]
    sr = np.zeros((NCORES, BPC, 128, 2, 968), bf16)
    sr[..., :961] = search.reshape(NCORES, BPC, 2, 128, 961).transpose(0, 1, 3, 2, 4).astype(bf16)
    # kernel [64,256,7,7] -> per core [128(ci), 2(cig), 8(s), 49]
    kr = np.zeros((NCORES, 128, 2, BPC, 52), bf16)
    kr[..., :49] = kern.reshape(NCORES, BPC, 2, 128, 49).transpose(0, 3, 2, 1, 4).astype(bf16)

    in_maps = []
    for c in range(NCORES):
        in_maps.append({
            "search": np.ascontiguousarray(sr[c]),
            "tmpl": np.ascontiguousarray(kr[c]),
            "wkt": wkt, "wst": wst, "w1t": w1t, "w2t": w2t,
            "bnk": bnk, "bns": bns, "bnh": bnh, "b2t": b2t, "ident": ident,
        })
    return in_maps


def get_program(bench_R=0):
    key = f"nc{bench_R}"
    if key not in _CACHE:
        _CACHE[key] = _build(bench_R)
    return _CACHE[key]


def kernel(**inputs):
    from concourse.bass_utils import run_bass_kernel_spmd
    nc = get_program()
    in_maps = _pack(inputs)
    res = run_bass_kernel_spmd(nc, in_maps, core_ids=list(range(NCORES)))
    out = np.stack([res.results[c]["out"] for c in range(NCORES)], axis=0)
    return out.reshape(64, 20, 25, 25).astype(np.float32)
